# revision 5
# baseline (speedup 1.0000x reference)
"""Self-contained Trainium2 Bass kernel for nn_BipartiteGNN (v2).

Collapsed-linear formulation: the network is fully linear, so the [1,1]
output reduces to degree-chain vectors (d = A 1, p = A d_other, z = A p_other
per side) contracted with the node features. The device computes the chains
with a local_scatter permutation network (no ap_gather):
  u-slab DMA -> LS-place -> scan-expand -> LS1 -> PE blockwise transpose ->
  LS2 -> strided tensor_reduce, all bf16 with fp32 accumulation; final
  Y = [mask,d,p,z] @ [X,1] via PE matmul; tiny weight recursion on host.
"""
import numpy as np

NCORES = 8
CH = 12544          # rows per NC (98*128)
NPAD = NCORES * CH  # 100352
NREAL = 100_000
SLAB = 784          # columns per partition (128*784 = NPAD)
NSLOT = 98          # rows per partition
LSMAX = 2046        # local_scatter dest limit (int16 elems)
BLK = 15            # mid blocks per LS1 chunk (15*128=1920 <= 2046)
L = 3


def map_ids(n):
    c = n // 12500
    return c * CH + (n - c * 12500)


def _rank_in_groups(keys, order_by):
    """rank of each element within its key group, ordered by order_by."""
    n = len(keys)
    order = np.lexsort((order_by, keys))
    ks = keys[order]
    first = np.ones(n, bool)
    first[1:] = ks[1:] != ks[:-1]
    gs = np.zeros(n, np.int64)
    idx = np.arange(n)
    gs[first] = idx[first]
    gs = np.maximum.accumulate(gs)
    rank_sorted = idx - gs
    rank = np.empty(n, np.int64)
    rank[order] = rank_sorted
    return rank


def build_scramble(rows_pad):
    """deg[NPAD], pos[NPAD] (padded id -> scrambled global pos),
    degsorted[NCORES, CH] (per-NC degrees in rank order)."""
    deg = np.bincount(rows_pad, minlength=NPAD)
    pos = np.zeros(NPAD, np.int64)
    degsorted = np.zeros((NCORES, CH), np.int64)
    for c in range(NCORES):
        d = deg[c * CH:(c + 1) * CH]
        order = np.lexsort((np.arange(CH), -d))
        rank = np.empty(CH, np.int64)
        rank[order] = np.arange(CH)
        part = rank % 128
        slot = rank // 128
        pos[c * CH:(c + 1) * CH] = c * CH + part * NSLOT + slot
        degsorted[c] = d[order]
    return deg, pos, degsorted


def rebalance_side(pos, rows_pad, cols_pad, col_pos):
    """Permute rows among the 8 partitions of each slab-group (same slot)
    to flatten the (col-partition q, row-partition p') flow matrix.
    Leaves this side's slab map (= what the other direction reads) intact:
    slab((8g+i)*98+slot) == g for any i in [0,8)."""
    q_edge = (col_pos[cols_pad] // SLAB).astype(np.int32)
    newpos = pos.copy()
    for c in range(NCORES):
        base_id = c * CH
        loc = pos[base_id:base_id + CH] - c * CH
        part = loc // NSLOT
        slot = loc % NSLOT
        grp = part // 8
        sel = np.where((rows_pad >= base_id) & (rows_pad < base_id + CH))[0]
        er = (rows_pad[sel] - base_id).astype(np.int64)
        eq = q_edge[sel]
        order = np.argsort(er, kind="stable")
        er_s = er[order]
        eq_s = eq[order]
        starts = np.searchsorted(er_s, np.arange(CH + 1))
        deg = starts[1:] - starts[:-1]
        cnt = np.zeros((16, 128, 8), np.int32)
        ordr = np.lexsort((-deg, slot, grp))
        qrows = [None] * CH
        pick_of = np.zeros(CH, np.int8)
        for rid in range(CH):
            qs = eq_s[starts[rid]:starts[rid + 1]]
            if len(qs):
                uc = np.bincount(qs, minlength=128)
                qrows[rid] = np.nonzero(uc)[0], uc
        for pass_i in range(2):
            for blk_i in range(0, CH, 8):
                ids = ordr[blk_i:blk_i + 8]
                g = grp[ids[0]]
                s_ = slot[ids[0]]
                cg = cnt[g]
                if pass_i == 1:
                    # remove this block's contributions, then re-pick all 8
                    for rid in ids:
                        if qrows[rid] is not None:
                            uq, uc = qrows[rid]
                            cg[uq, pick_of[rid]] -= uc[uq]
                used = np.zeros(8, bool)
                for rid in ids:
                    if qrows[rid] is not None:
                        uq, uc = qrows[rid]
                        sc = (cg[uq, :] + uc[uq, None]).max(axis=0)
                    else:
                        uq = None
                        sc = np.zeros(8, np.int64)
                    sc = np.where(used, 1 << 30, sc)
                    pick = int(np.argmin(sc))
                    used[pick] = True
                    pick_of[rid] = pick
                    if uq is not None:
                        cg[uq, pick] += uc[uq]
        for rid in range(CH):
            g = grp[rid]
            s_ = slot[rid]
            newpos[base_id + rid] = base_id + (8 * g + int(pick_of[rid])) * NSLOT + s_
    return newpos


def build_regions(degsorted_list):
    """Shared super-region table from per-NC/side degree-rank profiles.
    Micro-region b (slot b): L_micro[b] = max over profiles of
    max(deg[rank 128b .. 128b+127]) = deg at rank 128b (sorted desc).
    Merge consecutive micros into supers (cap padding).
    Returns list of (Lpad, cnt, rv_off, slot0) and RVLEN."""
    lm = np.zeros(NSLOT, np.int64)
    for ds in degsorted_list:
        lm = np.maximum(lm, ds[:, ::1].reshape(NCORES, NSLOT, 128).max(axis=2).max(axis=0))
    # ensure even L (AP niceness) and >=2
    supers = []
    b = 0
    while b < NSLOT:
        Lmax = lm[b]
        e = b + 1
        while e < NSLOT and e - b < 24 and lm[e] >= max(2, Lmax * 0.93):
            e += 1
        Lpad = int(max(2, Lmax + (Lmax & 1)))
        supers.append([Lpad, e - b, 0, b])
        b = e
    off = 0
    for s in supers:
        s[2] = off
        off += s[0] * s[1]
    RVLEN = off + (off & 1)
    return supers, RVLEN


def build_direction(rows_pad, cols_pad, row_pos, col_pos, supers):
    """Per-NC edge bookkeeping for one direction."""
    rpos = row_pos[rows_pad]
    nc_of = rpos // CH
    rloc = rpos - nc_of * CH
    rpart = rloc // NSLOT
    rslot = rloc - rpart * NSLOT

    cpos = col_pos[cols_pad]
    cslab = cpos // SLAB
    cwithin = cpos - cslab * SLAB

    slot_off = np.zeros(NSLOT, np.int64)
    for (Lp, cnt, off, slot0) in supers:
        for k in range(cnt):
            slot_off[slot0 + k] = off + k * Lp

    cores = []
    for c in range(NCORES):
        sel = np.where(nc_of == c)[0]
        rp = rpart[sel]
        rs = rslot[sel]
        q = cslab[sel]
        cw = cwithin[sel]
        j = _rank_in_groups(rp * NSLOT + rs, np.arange(len(sel)))
        t = slot_off[rs] + j                       # RV position within p'

        # col side: EV layout per partition q: per distinct col (ascending):
        # [start slot][edge slots]; compute ev index per edge + run starts.
        key_c = q * SLAB + cw
        order_c = np.lexsort((np.arange(len(sel)), key_c))
        kc_s = key_c[order_c]
        firstc = np.ones(len(sel), bool)
        firstc[1:] = kc_s[1:] != kc_s[:-1]
        qq = kc_s // SLAB
        nslots_sorted = firstc.astype(np.int64) + 1
        cs = np.cumsum(nslots_sorted)
        qfirst = np.ones(len(sel), bool)
        qfirst[1:] = qq[1:] != qq[:-1]
        base = np.zeros(len(sel), np.int64)
        base[qfirst] = cs[qfirst] - nslots_sorted[qfirst]
        base = np.maximum.accumulate(base)
        evpos_sorted = cs - base - 1
        ev_i = np.empty(len(sel), np.int64)
        ev_i[order_c] = evpos_sorted
        ev_len = np.zeros(128, np.int64)
        if len(sel):
            lastq = np.ones(len(sel), bool)
            lastq[:-1] = qq[1:] != qq[:-1]
            ev_len[qq[lastq]] = evpos_sorted[lastq] + 1
        rs_q = qq[firstc]
        rs_cw = kc_s[firstc] - rs_q * SLAB
        rs_pos = evpos_sorted[firstc] - 1
        cores.append(dict(rp=rp, t=t, q=q, ev_i=ev_i, ev_len=ev_len,
                          rs_q=rs_q, rs_cw=rs_cw, rs_pos=rs_pos))
    return cores


def finalize_direction(cores, FC, K, RVLEN):
    """Device arrays per core given shared sizes."""
    nch1 = (K + BLK - 1) // BLK
    MIDW = K * 128
    nch2 = int(np.ceil(RVLEN / LSMAX))
    out = []
    for co in cores:
        rp, t, q, ev_i = co["rp"], co["t"], co["q"], co["ev_i"]
        b = _rank_in_groups(q * 128 + rp, t)
        assert b.max(initial=0) < K

        # LS-place idx, chunked over EV dest (chunks of LSMAX)
        nchp = int(np.ceil(FC / LSMAX))
        lsp = -np.ones((nchp, 128, SLAB), np.int16)
        ck = co["rs_pos"] // LSMAX
        lsp[ck, co["rs_q"], co["rs_cw"]] = (co["rs_pos"] - ck * LSMAX).astype(np.int16)

        m = np.ones((128, FC), np.float32)
        m[co["rs_q"], co["rs_pos"]] = 0.0

        ls1 = -np.ones((nch1, 128, FC), np.int16)
        ci = b // BLK
        dest1 = (b - ci * BLK) * 128 + rp
        ls1[ci, q, ev_i] = dest1.astype(np.int16)

        # midT pos of edge: (rp, b*128 + q)
        ls2 = -np.ones((nch2, 128, MIDW), np.int16)
        cj = t // LSMAX
        dest2 = t - cj * LSMAX
        ls2[cj, rp, b * 128 + q] = dest2.astype(np.int16)

        dmask = np.zeros((128, RVLEN), np.float32)
        dmask[rp, t] = 1.0

        out.append(dict(lsp=lsp, m=m, ls1=ls1, ls2=ls2, dmask=dmask,
                        rp=rp, t=t, q=q, b=b, ev_i=ev_i))
    return out, nch1, nch2, MIDW


def _bf16(x):
    return x.astype(np.float32).view(np.uint32) >> 16


def to_bf16_f32(x):
    """round-to-nearest-even bf16, kept as float32."""
    x = np.asarray(x, np.float32)
    u = x.view(np.uint32)
    rounded = ((u + 0x7FFF + ((u >> 16) & 1)) & 0xFFFF0000).astype(np.uint32)
    return rounded.view(np.float32)


def mirror_pass(fin, uslab, FC, MIDW, RVLEN, supers, with_bf16=True):
    """Numpy mirror of one core's pass. uslab [128, SLAB] f32 (already the
    slab contents). Returns OUT [128, NSLOT] f32."""
    conv = to_bf16_f32 if with_bf16 else (lambda x: x)
    uslab = conv(uslab)
    EV = np.zeros((128, FC), np.float32)
    lsp = fin["lsp"].astype(np.int64)
    for ck in range(lsp.shape[0]):
        pok, sok = np.where(lsp[ck] >= 0)
        EV[pok, ck * LSMAX + lsp[ck, pok, sok]] = uslab[pok, sok]
    # segmented scan (expand): m=0 starts a new segment with value EV
    m = fin["m"]
    bidx = np.where(m == 0.0, np.arange(FC)[None, :], -1)
    bidx = np.maximum.accumulate(bidx, axis=1)
    sc = np.take_along_axis(EV, np.maximum(bidx, 0), axis=1)
    sc[bidx < 0] = 0.0
    # LS1 -> mid -> transpose -> midT  (pure permutation; emulate directly)
    midT = np.zeros((128, MIDW), np.float32)
    rp, t, q, b, ev_i = (fin[k] for k in ("rp", "t", "q", "b", "ev_i"))
    midT[rp, b * 128 + q] = sc[q, ev_i]
    RV = np.zeros((128, RVLEN), np.float32)
    RV[rp, t] = midT[rp, b * 128 + q]
    OUT = np.zeros((128, NSLOT), np.float32)
    for (Lp, cnt, off, slot0) in supers:
        seg = RV[:, off:off + cnt * Lp].reshape(128, cnt, Lp)
        OUT[:, slot0:slot0 + cnt] = seg.sum(axis=2)
    return OUT


def build_all(edges_A, edges_B):
    """edges_A = edges_s2t (rows=s=row0, cols=t=row1); edges_B = edges_t2s."""
    rowsA = map_ids(np.asarray(edges_A[0], np.int64))
    colsA = map_ids(np.asarray(edges_A[1], np.int64))
    rowsB = map_ids(np.asarray(edges_B[0], np.int64))
    colsB = map_ids(np.asarray(edges_B[1], np.int64))

    degA, posS, dsrtA = build_scramble(rowsA)   # side s scramble from A rows
    degB, posT, dsrtB = build_scramble(rowsB)   # side t scramble from B rows
    posS = rebalance_side(posS, rowsA, colsA, posT)
    posT = rebalance_side(posT, rowsB, colsB, posS)
    supers, RVLEN = build_regions([dsrtA, dsrtB])

    coresA = build_direction(rowsA, colsA, posS, posT, supers)
    coresB = build_direction(rowsB, colsB, posT, posS, supers)

    FC = 0
    for co in coresA + coresB:
        FC = max(FC, int(co["ev_len"].max()))
    FC += FC & 1
    K = 0
    for co in coresA + coresB:
        b = _rank_in_groups(co["q"] * 128 + co["rp"], co["t"])
        K = max(K, int(b.max(initial=0)) + 1)

    finA, nch1, nch2, MIDW = finalize_direction(coresA, FC, K, RVLEN)
    finB, _, _, _ = finalize_direction(coresB, FC, K, RVLEN)

    return dict(finA=finA, finB=finB, posS=posS, posT=posT,
                supers=supers, RVLEN=RVLEN, FC=FC, K=K,
                nch1=nch1, nch2=nch2, MIDW=MIDW)


def gpos_to_slab(g):
    """global scrambled array [NPAD] -> [128, SLAB] slab view."""
    return g.reshape(128, SLAB)


def full_numpy(inputs, lay=None, with_bf16=True):
    """End-to-end mirror: d,p,z chains + final Y/S + recursion."""
    if lay is None:
        lay = build_all(np.asarray(inputs["edges_s2t"], np.int64),
                        np.asarray(inputs["edges_t2s"], np.int64))
    supers, RVLEN, FC, MIDW = (lay[k] for k in ("supers", "RVLEN", "FC", "MIDW"))
    conv = to_bf16_f32 if with_bf16 else (lambda x: x)

    def run_chain(fins, u_global):
        """one direction pass for all cores; u_global [NPAD] f32 scrambled
        (other side's order); returns this side's outputs [NPAD] scrambled."""
        out = np.zeros(NPAD, np.float32)
        us = gpos_to_slab(conv(u_global))
        for c in range(NCORES):
            O = mirror_pass(fins[c], us, FC, MIDW, RVLEN, supers, with_bf16)
            out[c * CH:(c + 1) * CH] = O.reshape(-1)  # p*98+slot partition-major
        return out

    def d_chain(fins):
        out = np.zeros(NPAD, np.float32)
        for c in range(NCORES):
            RV = fins[c]["dmask"]
            O = np.zeros((128, NSLOT), np.float32)
            for (Lp, cnt, off, slot0) in supers:
                O[:, slot0:slot0 + cnt] = RV[:, off:off + cnt * Lp].reshape(
                    128, cnt, Lp).sum(axis=2)
            out[c * CH:(c + 1) * CH] = O.reshape(-1)
        return out

    finA, finB = lay["finA"], lay["finB"]
    d_s = d_chain(finA)
    d_t = d_chain(finB)
    p_s = run_chain(finA, d_t)
    p_t = run_chain(finB, d_s)
    z_s = run_chain(finA, p_t)
    z_t = run_chain(finB, p_s)

    # final: Y = U4 @ [X, 1] per side, in scrambled order
    xs = pack_x_scrambled(np.asarray(inputs["x_s"], np.float32), lay["posS"])
    xt = pack_x_scrambled(np.asarray(inputs["x_t"], np.float32), lay["posT"])
    mask_s = mask_scrambled(lay["posS"])
    mask_t = mask_scrambled(lay["posT"])
    Us = np.stack([mask_s, d_s * mask_s, p_s * mask_s, z_s * mask_s])
    Ut = np.stack([mask_t, d_t * mask_t, p_t * mask_t, z_t * mask_t])
    Ys = Us @ xs
    Yt = Ut @ xt
    Ss = Us.sum(1)
    St = Ut.sum(1)
    return final_recursion(Ys, Yt, Ss, St, inputs)


def pack_x_scrambled(x, pos):
    out = np.zeros((NPAD, x.shape[1]), np.float32)
    out[pos[map_ids(np.arange(NREAL))]] = x
    return out


def mask_scrambled(pos):
    m = np.zeros(NPAD, np.float32)
    m[pos[map_ids(np.arange(NREAL))]] = 1.0
    return m


def final_recursion(Ys, Yt, Ss, St, inputs):
    f64 = np.float64
    Wl_s2t = np.asarray(inputs["Wl_s2t"], f64); Wr_s2t = np.asarray(inputs["Wr_s2t"], f64)
    b_s2t = np.asarray(inputs["b_s2t"], f64)
    Wl_t2s = np.asarray(inputs["Wl_t2s"], f64); Wr_t2s = np.asarray(inputs["Wr_t2s"], f64)
    b_t2s = np.asarray(inputs["b_t2s"], f64)
    W_lin = np.asarray(inputs["W_lin"], f64); b_lin = np.asarray(inputs["b_lin"], f64)
    Ys = Ys.astype(f64); Yt = Yt.astype(f64)
    Ss = Ss.astype(f64); St = St.astype(f64)

    def term(side, u_id, r, layer):
        if layer == 0:
            Y = Ys if side == "s" else Yt
            return Y[u_id] @ r
        if side == "s":
            Wl, Wr, bb, S, other = Wl_t2s[layer-1], Wr_t2s[layer-1], b_t2s[layer-1], Ss, "t"
        else:
            Wl, Wr, bb, S, other = Wl_s2t[layer-1], Wr_s2t[layer-1], b_s2t[layer-1], St, "s"
        return (term(other, u_id + 1, Wl @ r, layer - 1)
                + S[u_id] * (bb @ r)
                + term(side, u_id, Wr @ r, layer - 1))

    r0 = W_lin[:, 0]
    tot = term("s", 0, r0, L) + term("t", 0, r0, L) + b_lin[0]
    return np.array([[tot]], dtype=np.float32)


# ---------------- device kernel ----------------
from contextlib import ExitStack
import concourse.bass as bass
import concourse.tile as tile
from concourse import bacc, mybir
from concourse.bass_utils import run_bass_kernel_spmd
from concourse.masks import make_identity
import ml_dtypes

F32 = mybir.dt.float32
BF16 = mybir.dt.bfloat16
I16 = mybir.dt.int16


def build_kernel_v2(FC, K, nch1, nch2, nchp, RVLEN, supers, reps=1,
                    scan_bf16=True, dbg=False):
    MIDW = K * 128
    nc = bacc.Bacc("TRN2", target_bir_lowering=False, debug=False,
                   num_devices=8)

    def din(name, shape, dt=F32):
        return nc.dram_tensor(name, shape, dt, kind="ExternalInput")

    ins = {}
    for D in ("A", "B"):
        ins[f"lsp{D}"] = din(f"lsp{D}", [128, nchp * SLAB], I16)
        ins[f"m{D}"] = din(f"m{D}", [128, FC], BF16 if scan_bf16 else F32)
        ins[f"ls1{D}"] = din(f"ls1{D}", [128, nch1 * FC], I16)
        ins[f"ls2{D}"] = din(f"ls2{D}", [128, nch2 * MIDW], I16)
        ins[f"dmask{D}"] = din(f"dmask{D}", [128, RVLEN], BF16)
    ins["xs"] = din("xs", [CH, 64])
    ins["xt"] = din("xt", [CH, 64])
    ins["rmask_s"] = din("rmask_s", [CH])
    ins["rmask_t"] = din("rmask_t", [CH])

    res_s = nc.dram_tensor("res_s", [4, 65], F32, kind="ExternalOutput")
    res_t = nc.dram_tensor("res_t", [4, 65], F32, kind="ExternalOutput")

    dram = {}
    dram["d_loc2"] = nc.dram_tensor("d_loc2", [2 * CH], F32)
    dram["p_loc2"] = nc.dram_tensor("p_loc2", [2 * CH], F32)
    dram["z_locA"] = nc.dram_tensor("z_locA", [CH], F32)
    dram["z_locB"] = nc.dram_tensor("z_locB", [CH], F32)
    if dbg:
        dbg_cp = {}
        for nm in ("d_loc2", "p_loc2", "z_locA", "z_locB"):
            dbg_cp[nm] = nc.dram_tensor("dbg_" + nm, list(dram[nm].shape),
                                        F32, kind="ExternalOutput")
        dbg_sc = nc.dram_tensor("dbg_sc", [128, FC], BF16,
                                kind="ExternalOutput")
        dbg_mid = nc.dram_tensor("dbg_mid", [128, MIDW], BF16,
                                 kind="ExternalOutput")
        dbg_midT = nc.dram_tensor("dbg_midT", [128, MIDW], BF16,
                                  kind="ExternalOutput")
        dbg_rv = nc.dram_tensor("dbg_rv", [128, RVLEN], BF16,
                                kind="ExternalOutput")
    dram["d_full2"] = nc.dram_tensor("d_full2", [2 * NPAD], F32,
                                     addr_space="Shared")
    dram["p_full2"] = nc.dram_tensor("p_full2", [2 * NPAD], F32,
                                     addr_space="Shared")

    with tile.TileContext(nc) as tc, ExitStack() as ctx:
        stat = ctx.enter_context(tc.tile_pool(name="stat", bufs=1))
        idxp = ctx.enter_context(tc.tile_pool(name="idxp", bufs=1))
        wb = ctx.enter_context(tc.tile_pool(name="wb", bufs=1))
        ws = ctx.enter_context(tc.tile_pool(name="ws", bufs=2))
        psum = ctx.enter_context(tc.tile_pool(name="ps", bufs=2, space="PSUM"))

        ident = stat.tile([128, 128], BF16, tag="id")
        make_identity(nc, ident[:])

        statics = {}
        for D in ("A", "B"):
            t = stat.tile([128, nchp * SLAB], I16, tag=f"lsp{D}")
            nc.sync.dma_start(t[:], ins[f"lsp{D}"].ap())
            statics[f"lsp{D}"] = t
            t = stat.tile([128, FC], BF16 if scan_bf16 else F32, tag=f"m{D}")
            nc.sync.dma_start(t[:], ins[f"m{D}"].ap())
            statics[f"m{D}"] = t
            t = stat.tile([128, RVLEN], BF16, tag=f"dm{D}")
            nc.sync.dma_start(t[:], ins[f"dmask{D}"].ap())
            statics[f"dm{D}"] = t

        def reduce_out(rv_ap_tile, out_dram, out_off, in_bf16=True):
            OUT = ws.tile([128, NSLOT], F32, tag="OUT")
            for (Lp, cnt, off_rv, slot0) in supers:
                nc.vector.tensor_reduce(
                    out=OUT[:, slot0:slot0 + cnt],
                    in_=bass.AP(rv_ap_tile.tensor, off_rv,
                                [[RVLEN, 128], [Lp, cnt], [1, Lp]]),
                    axis=mybir.AxisListType.X, op=mybir.AluOpType.add)
            nc.sync.dma_start(
                bass.AP(out_dram, out_off, [[NSLOT, 128], [1, NSLOT]]),
                OUT[:])

        def pass_dir(D, table_dram, side_off, out_dram, out_off):
            # u-slab [128, 784] f32 -> bf16
            us32 = ws.tile([128, SLAB], F32, tag="us32")
            nc.sync.dma_start(
                us32[:],
                bass.AP(table_dram, side_off,
                        [[2 * CH, 8], [SLAB, 16], [1, SLAB]]))
            us16 = ws.tile([128, SLAB], BF16, tag="us16")
            nc.vector.tensor_copy(us16[:], us32[:])
            # LS-place into EV chunks
            EV = wb.tile([128, FC], BF16, tag="EV")
            lsp = statics[f"lsp{D}"]
            for ck in range(nchp):
                o = ck * LSMAX
                ln = min(LSMAX, FC - o)
                nc.gpsimd.local_scatter(
                    EV[:, o:o + ln], us16[:],
                    lsp[:, ck * SLAB:(ck + 1) * SLAB],
                    channels=128, num_elems=ln, num_idxs=SLAB)
            # scan-expand
            if scan_bf16:
                sc = wb.tile([128, FC], BF16, tag="sc")
                nc.vector.tensor_tensor_scan(
                    sc[:], statics[f"m{D}"][:], EV[:], 0.0,
                    mybir.AluOpType.mult, mybir.AluOpType.add)
            else:
                EV32 = wb.tile([128, FC], F32, tag="EV32")
                nc.vector.tensor_copy(EV32[:], EV[:])
                sc32 = wb.tile([128, FC], F32, tag="sc32")
                nc.vector.tensor_tensor_scan(
                    sc32[:], statics[f"m{D}"][:], EV32[:], 0.0,
                    mybir.AluOpType.mult, mybir.AluOpType.add)
                sc = wb.tile([128, FC], BF16, tag="sc")
                nc.vector.tensor_copy(sc[:], sc32[:])
            # LS1 -> mid
            ls1 = idxp.tile([128, nch1 * FC], I16, tag="ls1")
            nc.sync.dma_start(ls1[:], ins[f"ls1{D}"].ap())
            mid = wb.tile([128, MIDW], BF16, tag="mid")
            for ci in range(nch1):
                o = ci * BLK * 128
                ln = min(BLK * 128, MIDW - o)
                nc.gpsimd.local_scatter(
                    mid[:, o:o + ln], sc[:],
                    ls1[:, ci * FC:(ci + 1) * FC],
                    channels=128, num_elems=ln, num_idxs=FC)
            # transpose blocks (groups of 4 into one PSUM bank)
            midT = wb.tile([128, MIDW], BF16, tag="midT")
            for g in range(0, K, 4):
                nb = min(4, K - g)
                ps = psum.tile([128, 512], BF16, tag="tps")
                for bi in range(nb):
                    b = g + bi
                    nc.tensor.transpose(
                        out=ps[:, bi * 128:(bi + 1) * 128],
                        in_=mid[:, b * 128:(b + 1) * 128],
                        identity=ident[:])
                nc.vector.tensor_copy(
                    midT[:, g * 128:(g + nb) * 128], ps[:, :nb * 128])
            # LS2 -> RV
            ls2 = idxp.tile([128, nch2 * MIDW], I16, tag="ls2")
            nc.sync.dma_start(ls2[:], ins[f"ls2{D}"].ap())
            RV = wb.tile([128, RVLEN], BF16, tag="RV")
            for cj in range(nch2):
                o = cj * LSMAX
                ln = min(LSMAX, RVLEN - o)
                nc.gpsimd.local_scatter(
                    RV[:, o:o + ln], midT[:],
                    ls2[:, cj * MIDW:(cj + 1) * MIDW],
                    channels=128, num_elems=ln, num_idxs=MIDW)
            if dbg and D == "A" and out_dram is dram["p_loc2"]:
                nc.sync.dma_start(dbg_sc.ap(), sc[:])
                nc.sync.dma_start(dbg_mid.ap(), mid[:])
                nc.sync.dma_start(dbg_midT.ap(), midT[:])
                nc.sync.dma_start(dbg_rv.ap(), RV[:])
            reduce_out(RV, out_dram, out_off)

        def allgather(loc, full):
            nc.gpsimd.collective_compute(
                "AllGather", mybir.AluOpType.bypass,
                replica_groups=[list(range(8))],
                ins=[bass.AP(loc, 0, [[1, 1], [1, 2 * CH]]).opt()],
                outs=[bass.AP(full, 0, [[1, 1], [1, 2 * NPAD]]).opt()])

        for _ in range(reps):
            # d phase: reduce the static dmask
            reduce_out(statics["dmA"], dram["d_loc2"], 0)
            reduce_out(statics["dmB"], dram["d_loc2"], CH)
            allgather(dram["d_loc2"], dram["d_full2"])
            # p phase: A consumes side t (off CH), B consumes side s (off 0)
            pass_dir("A", dram["d_full2"], CH, dram["p_loc2"], 0)
            pass_dir("B", dram["d_full2"], 0, dram["p_loc2"], CH)
            allgather(dram["p_loc2"], dram["p_full2"])
            # z phase
            pass_dir("A", dram["p_full2"], CH, dram["z_locA"], 0)
            pass_dir("B", dram["p_full2"], 0, dram["z_locB"], 0)

        if dbg:
            for nm in ("d_loc2", "p_loc2", "z_locA", "z_locB"):
                n_el = dram[nm].shape[0]
                nc.sync.dma_start(
                    bass.AP(dbg_cp[nm], 0, [[1, 1], [1, n_el]]),
                    bass.AP(dram[nm], 0, [[1, 1], [1, n_el]]))

        # final: per side Y[4,65] = sum_n u4[n] * [X[n,:], 1]
        for side, xin, off, zl, rout in (
                ("s", "xs", 0, "z_locA", res_s),
                ("t", "xt", CH, "z_locB", res_t)):
            rmask_in = ins[f"rmask_{side}"]
            xr = idxp.tile([128, NSLOT, 65], F32, tag="xr")
            nc.sync.dma_start(
                bass.AP(xr.tensor, 0, [[NSLOT * 65, 128], [65, NSLOT], [1, 64]]),
                ins[xin].ap())
            nc.vector.memset(
                bass.AP(xr.tensor, 64, [[NSLOT * 65, 128], [65, NSLOT], [1, 1]]),
                1.0)
            u4 = ws.tile([128, NSLOT, 4], F32, tag="u4")
            nc.sync.dma_start(
                bass.AP(u4.tensor, 0, [[NSLOT * 4, 128], [4, NSLOT], [1, 1]]),
                rmask_in.ap())
            for i, (dr, doff) in enumerate(((dram["d_loc2"], off),
                                            (dram["p_loc2"], off),
                                            (dram[zl], 0))):
                nc.sync.dma_start(
                    bass.AP(u4.tensor, i + 1, [[NSLOT * 4, 128], [4, NSLOT], [1, 1]]),
                    bass.AP(dr, doff, [[NSLOT, 128], [1, NSLOT]]))
            ps = psum.tile([4, 65], F32, tag="fps")
            for j in range(NSLOT):
                nc.tensor.matmul(ps[:], u4[:, j, :], xr[:, j, :],
                                 start=(j == 0), stop=(j == NSLOT - 1))
            outt = ws.tile([4, 65], F32, tag="outt")
            nc.vector.tensor_copy(outt[:], ps[:])
            nc.sync.dma_start(rout.ap(), outt[:])

    nc.compile()
    return nc


def _to_bf16(x):
    return np.asarray(x, np.float32).astype(ml_dtypes.bfloat16)


def make_in_maps(lay, inputs):
    xs = pack_x_scrambled(np.asarray(inputs["x_s"], np.float32), lay["posS"])
    xt = pack_x_scrambled(np.asarray(inputs["x_t"], np.float32), lay["posT"])
    rmask_s = mask_scrambled(lay["posS"])
    rmask_t = mask_scrambled(lay["posT"])
    nchp = lay["finA"][0]["lsp"].shape[0]
    scan_bf16 = lay.get("scan_bf16", True)
    in_maps = []
    for c in range(NCORES):
        im = {}
        for D, fins in (("A", lay["finA"]), ("B", lay["finB"])):
            f = fins[c]
            im[f"lsp{D}"] = np.ascontiguousarray(
                f["lsp"].transpose(1, 0, 2).reshape(128, -1))
            im[f"m{D}"] = (_to_bf16(f["m"]) if scan_bf16
                           else np.asarray(f["m"], np.float32))
            im[f"ls1{D}"] = np.ascontiguousarray(
                f["ls1"].transpose(1, 0, 2).reshape(128, -1))
            im[f"ls2{D}"] = np.ascontiguousarray(
                f["ls2"].transpose(1, 0, 2).reshape(128, -1))
            im[f"dmask{D}"] = _to_bf16(f["dmask"])
        im["xs"] = xs[c * CH:(c + 1) * CH]
        im["xt"] = xt[c * CH:(c + 1) * CH]
        im["rmask_s"] = rmask_s[c * CH:(c + 1) * CH]
        im["rmask_t"] = rmask_t[c * CH:(c + 1) * CH]
        in_maps.append(im)
    return in_maps


_NC_CACHE = {}


def prepare_for_bench(inputs):
    lay = build_all(np.asarray(inputs["edges_s2t"], np.int64),
                    np.asarray(inputs["edges_t2s"], np.int64))
    in_maps = make_in_maps(lay, inputs)
    return dict(lay=lay, in_maps=in_maps)


def build_from_prep(prep, reps=1):
    lay = prep["lay"]
    nchp = lay["finA"][0]["lsp"].shape[0]
    return build_kernel_v2(lay["FC"], lay["K"], lay["nch1"], lay["nch2"],
                           nchp, lay["RVLEN"], lay["supers"], reps=reps)


def kernel(**inputs) -> np.ndarray:
    prep = prepare_for_bench(inputs)
    lay = prep["lay"]
    nchp = lay["finA"][0]["lsp"].shape[0]
    key = (lay["FC"], lay["K"], lay["nch1"], lay["nch2"], nchp,
           lay["RVLEN"], tuple(tuple(s) for s in lay["supers"]))
    if key not in _NC_CACHE:
        _NC_CACHE[key] = build_kernel_v2(
            lay["FC"], lay["K"], lay["nch1"], lay["nch2"], nchp,
            lay["RVLEN"], lay["supers"])
    nc = _NC_CACHE[key]
    res = run_bass_kernel_spmd(nc, prep["in_maps"], core_ids=list(range(8)),
                               trace=False)
    Ys = sum(r["res_s"] for r in res.results)
    Yt = sum(r["res_t"] for r in res.results)
    return final_recursion(Ys[:, :64], Yt[:, :64], Ys[:, 64], Yt[:, 64],
                           inputs)




# revision 6
# speedup vs baseline: 1.0425x; 1.0425x over previous
"""Self-contained Trainium2 Bass kernel for nn_BipartiteGNN (v2).

Collapsed-linear formulation: the network is fully linear, so the [1,1]
output reduces to degree-chain vectors (d = A 1, p = A d_other, z = A p_other
per side) contracted with the node features. The device computes the chains
with a local_scatter permutation network (no ap_gather):
  u-slab DMA -> LS-place -> scan-expand -> LS1 -> PE blockwise transpose ->
  LS2 -> strided tensor_reduce, all bf16 with fp32 accumulation; final
  Y = [mask,d,p,z] @ [X,1] via PE matmul; tiny weight recursion on host.
"""
import numpy as np

NCORES = 8
CH = 12544          # rows per NC (98*128)
NPAD = NCORES * CH  # 100352
NREAL = 100_000
SLAB = 784          # columns per partition (128*784 = NPAD)
NSLOT = 98          # rows per partition
LSMAX = 2046        # local_scatter dest limit (int16 elems)
BLK = 15            # mid blocks per LS1 chunk (15*128=1920 <= 2046)
L = 3


def map_ids(n):
    c = n // 12500
    return c * CH + (n - c * 12500)


def _rank_in_groups(keys, order_by):
    """rank of each element within its key group, ordered by order_by."""
    n = len(keys)
    order = np.lexsort((order_by, keys))
    ks = keys[order]
    first = np.ones(n, bool)
    first[1:] = ks[1:] != ks[:-1]
    gs = np.zeros(n, np.int64)
    idx = np.arange(n)
    gs[first] = idx[first]
    gs = np.maximum.accumulate(gs)
    rank_sorted = idx - gs
    rank = np.empty(n, np.int64)
    rank[order] = rank_sorted
    return rank


def build_scramble(rows_pad):
    """deg[NPAD], pos[NPAD] (padded id -> scrambled global pos),
    degsorted[NCORES, CH] (per-NC degrees in rank order)."""
    deg = np.bincount(rows_pad, minlength=NPAD)
    pos = np.zeros(NPAD, np.int64)
    degsorted = np.zeros((NCORES, CH), np.int64)
    for c in range(NCORES):
        d = deg[c * CH:(c + 1) * CH]
        order = np.lexsort((np.arange(CH), -d))
        rank = np.empty(CH, np.int64)
        rank[order] = np.arange(CH)
        part = rank % 128
        slot = rank // 128
        pos[c * CH:(c + 1) * CH] = c * CH + part * NSLOT + slot
        degsorted[c] = d[order]
    return deg, pos, degsorted


def rebalance_side(pos, rows_pad, cols_pad, col_pos):
    """Permute rows among the 8 partitions of each slab-group (same slot)
    to flatten the (col-partition q, row-partition p') flow matrix.
    Leaves this side's slab map (= what the other direction reads) intact:
    slab((8g+i)*98+slot) == g for any i in [0,8)."""
    q_edge = (col_pos[cols_pad] // SLAB).astype(np.int32)
    newpos = pos.copy()
    for c in range(NCORES):
        base_id = c * CH
        loc = pos[base_id:base_id + CH] - c * CH
        part = loc // NSLOT
        slot = loc % NSLOT
        grp = part // 8
        sel = np.where((rows_pad >= base_id) & (rows_pad < base_id + CH))[0]
        er = (rows_pad[sel] - base_id).astype(np.int64)
        eq = q_edge[sel]
        order = np.argsort(er, kind="stable")
        er_s = er[order]
        eq_s = eq[order]
        starts = np.searchsorted(er_s, np.arange(CH + 1))
        deg = starts[1:] - starts[:-1]
        cnt = np.zeros((16, 128, 8), np.int32)
        ordr = np.lexsort((-deg, slot, grp))
        qrows = [None] * CH
        pick_of = np.zeros(CH, np.int8)
        for rid in range(CH):
            qs = eq_s[starts[rid]:starts[rid + 1]]
            if len(qs):
                uc = np.bincount(qs, minlength=128)
                qrows[rid] = np.nonzero(uc)[0], uc
        for pass_i in range(2):
            for blk_i in range(0, CH, 8):
                ids = ordr[blk_i:blk_i + 8]
                g = grp[ids[0]]
                s_ = slot[ids[0]]
                cg = cnt[g]
                if pass_i == 1:
                    # remove this block's contributions, then re-pick all 8
                    for rid in ids:
                        if qrows[rid] is not None:
                            uq, uc = qrows[rid]
                            cg[uq, pick_of[rid]] -= uc[uq]
                used = np.zeros(8, bool)
                for rid in ids:
                    if qrows[rid] is not None:
                        uq, uc = qrows[rid]
                        sc = (cg[uq, :] + uc[uq, None]).max(axis=0)
                    else:
                        uq = None
                        sc = np.zeros(8, np.int64)
                    sc = np.where(used, 1 << 30, sc)
                    pick = int(np.argmin(sc))
                    used[pick] = True
                    pick_of[rid] = pick
                    if uq is not None:
                        cg[uq, pick] += uc[uq]
        for rid in range(CH):
            g = grp[rid]
            s_ = slot[rid]
            newpos[base_id + rid] = base_id + (8 * g + int(pick_of[rid])) * NSLOT + s_
    return newpos


def build_regions(degsorted_list):
    """Shared super-region table from per-NC/side degree-rank profiles.
    Micro-region b (slot b): L_micro[b] = max over profiles of
    max(deg[rank 128b .. 128b+127]) = deg at rank 128b (sorted desc).
    Merge consecutive micros into supers (cap padding).
    Returns list of (Lpad, cnt, rv_off, slot0) and RVLEN."""
    lm = np.zeros(NSLOT, np.int64)
    for ds in degsorted_list:
        lm = np.maximum(lm, ds[:, ::1].reshape(NCORES, NSLOT, 128).max(axis=2).max(axis=0))
    # ensure even L (AP niceness) and >=2
    supers = []
    b = 0
    while b < NSLOT:
        Lmax = lm[b]
        e = b + 1
        while e < NSLOT and e - b < 24 and lm[e] >= max(2, Lmax * 0.93):
            e += 1
        Lpad = int(max(2, Lmax + (Lmax & 1)))
        supers.append([Lpad, e - b, 0, b])
        b = e
    off = 0
    for s in supers:
        s[2] = off
        off += s[0] * s[1]
    RVLEN = off + (off & 1)
    return supers, RVLEN


def build_direction(rows_pad, cols_pad, row_pos, col_pos, supers):
    """Per-NC edge bookkeeping for one direction."""
    rpos = row_pos[rows_pad]
    nc_of = rpos // CH
    rloc = rpos - nc_of * CH
    rpart = rloc // NSLOT
    rslot = rloc - rpart * NSLOT

    cpos = col_pos[cols_pad]
    cslab = cpos // SLAB
    cwithin = cpos - cslab * SLAB

    slot_off = np.zeros(NSLOT, np.int64)
    for (Lp, cnt, off, slot0) in supers:
        for k in range(cnt):
            slot_off[slot0 + k] = off + k * Lp

    cores = []
    for c in range(NCORES):
        sel = np.where(nc_of == c)[0]
        rp = rpart[sel]
        rs = rslot[sel]
        q = cslab[sel]
        cw = cwithin[sel]
        j = _rank_in_groups(rp * NSLOT + rs, np.arange(len(sel)))
        t = slot_off[rs] + j                       # RV position within p'

        # col side: EV layout per partition q: per distinct col (ascending):
        # [start slot][edge slots]; compute ev index per edge + run starts.
        key_c = q * SLAB + cw
        order_c = np.lexsort((np.arange(len(sel)), key_c))
        kc_s = key_c[order_c]
        firstc = np.ones(len(sel), bool)
        firstc[1:] = kc_s[1:] != kc_s[:-1]
        qq = kc_s // SLAB
        nslots_sorted = firstc.astype(np.int64) + 1
        cs = np.cumsum(nslots_sorted)
        qfirst = np.ones(len(sel), bool)
        qfirst[1:] = qq[1:] != qq[:-1]
        base = np.zeros(len(sel), np.int64)
        base[qfirst] = cs[qfirst] - nslots_sorted[qfirst]
        base = np.maximum.accumulate(base)
        evpos_sorted = cs - base - 1
        ev_i = np.empty(len(sel), np.int64)
        ev_i[order_c] = evpos_sorted
        ev_len = np.zeros(128, np.int64)
        if len(sel):
            lastq = np.ones(len(sel), bool)
            lastq[:-1] = qq[1:] != qq[:-1]
            ev_len[qq[lastq]] = evpos_sorted[lastq] + 1
        rs_q = qq[firstc]
        rs_cw = kc_s[firstc] - rs_q * SLAB
        rs_pos = evpos_sorted[firstc] - 1
        cores.append(dict(rp=rp, t=t, q=q, ev_i=ev_i, ev_len=ev_len,
                          rs_q=rs_q, rs_cw=rs_cw, rs_pos=rs_pos))
    return cores


def finalize_direction(cores, FC, K, RVLEN):
    """Device arrays per core given shared sizes."""
    nch1 = (K + BLK - 1) // BLK
    MIDW = K * 128
    nch2 = int(np.ceil(RVLEN / LSMAX))
    out = []
    for co in cores:
        rp, t, q, ev_i = co["rp"], co["t"], co["q"], co["ev_i"]
        b = _rank_in_groups(q * 128 + rp, t)
        assert b.max(initial=0) < K

        # LS-place idx, chunked over EV dest (chunks of LSMAX)
        nchp = int(np.ceil(FC / LSMAX))
        lsp = -np.ones((nchp, 128, SLAB), np.int16)
        ck = co["rs_pos"] // LSMAX
        lsp[ck, co["rs_q"], co["rs_cw"]] = (co["rs_pos"] - ck * LSMAX).astype(np.int16)

        m = np.ones((128, FC), np.float32)
        m[co["rs_q"], co["rs_pos"]] = 0.0

        ls1 = -np.ones((nch1, 128, FC), np.int16)
        ci = b // BLK
        dest1 = (b - ci * BLK) * 128 + rp
        ls1[ci, q, ev_i] = dest1.astype(np.int16)

        # midT pos of edge: (rp, b*128 + q)
        ls2 = -np.ones((nch2, 128, MIDW), np.int16)
        cj = t // LSMAX
        dest2 = t - cj * LSMAX
        ls2[cj, rp, b * 128 + q] = dest2.astype(np.int16)

        dmask = np.zeros((128, RVLEN), np.float32)
        dmask[rp, t] = 1.0

        out.append(dict(lsp=lsp, m=m, ls1=ls1, ls2=ls2, dmask=dmask,
                        rp=rp, t=t, q=q, b=b, ev_i=ev_i))
    return out, nch1, nch2, MIDW


def _bf16(x):
    return x.astype(np.float32).view(np.uint32) >> 16


def to_bf16_f32(x):
    """round-to-nearest-even bf16, kept as float32."""
    x = np.asarray(x, np.float32)
    u = x.view(np.uint32)
    rounded = ((u + 0x7FFF + ((u >> 16) & 1)) & 0xFFFF0000).astype(np.uint32)
    return rounded.view(np.float32)


def mirror_pass(fin, uslab, FC, MIDW, RVLEN, supers, with_bf16=True):
    """Numpy mirror of one core's pass. uslab [128, SLAB] f32 (already the
    slab contents). Returns OUT [128, NSLOT] f32."""
    conv = to_bf16_f32 if with_bf16 else (lambda x: x)
    uslab = conv(uslab)
    EV = np.zeros((128, FC), np.float32)
    lsp = fin["lsp"].astype(np.int64)
    for ck in range(lsp.shape[0]):
        pok, sok = np.where(lsp[ck] >= 0)
        EV[pok, ck * LSMAX + lsp[ck, pok, sok]] = uslab[pok, sok]
    # segmented scan (expand): m=0 starts a new segment with value EV
    m = fin["m"]
    bidx = np.where(m == 0.0, np.arange(FC)[None, :], -1)
    bidx = np.maximum.accumulate(bidx, axis=1)
    sc = np.take_along_axis(EV, np.maximum(bidx, 0), axis=1)
    sc[bidx < 0] = 0.0
    # LS1 -> mid -> transpose -> midT  (pure permutation; emulate directly)
    midT = np.zeros((128, MIDW), np.float32)
    rp, t, q, b, ev_i = (fin[k] for k in ("rp", "t", "q", "b", "ev_i"))
    midT[rp, b * 128 + q] = sc[q, ev_i]
    RV = np.zeros((128, RVLEN), np.float32)
    RV[rp, t] = midT[rp, b * 128 + q]
    OUT = np.zeros((128, NSLOT), np.float32)
    for (Lp, cnt, off, slot0) in supers:
        seg = RV[:, off:off + cnt * Lp].reshape(128, cnt, Lp)
        OUT[:, slot0:slot0 + cnt] = seg.sum(axis=2)
    return OUT


def build_all(edges_A, edges_B):
    """edges_A = edges_s2t (rows=s=row0, cols=t=row1); edges_B = edges_t2s."""
    rowsA = map_ids(np.asarray(edges_A[0], np.int64))
    colsA = map_ids(np.asarray(edges_A[1], np.int64))
    rowsB = map_ids(np.asarray(edges_B[0], np.int64))
    colsB = map_ids(np.asarray(edges_B[1], np.int64))

    degA, posS, dsrtA = build_scramble(rowsA)   # side s scramble from A rows
    degB, posT, dsrtB = build_scramble(rowsB)   # side t scramble from B rows
    posS = rebalance_side(posS, rowsA, colsA, posT)
    posT = rebalance_side(posT, rowsB, colsB, posS)
    supers, RVLEN = build_regions([dsrtA, dsrtB])

    coresA = build_direction(rowsA, colsA, posS, posT, supers)
    coresB = build_direction(rowsB, colsB, posT, posS, supers)

    FC = 0
    for co in coresA + coresB:
        FC = max(FC, int(co["ev_len"].max()))
    FC += FC & 1
    K = 0
    for co in coresA + coresB:
        b = _rank_in_groups(co["q"] * 128 + co["rp"], co["t"])
        K = max(K, int(b.max(initial=0)) + 1)

    finA, nch1, nch2, MIDW = finalize_direction(coresA, FC, K, RVLEN)
    finB, _, _, _ = finalize_direction(coresB, FC, K, RVLEN)

    return dict(finA=finA, finB=finB, posS=posS, posT=posT,
                supers=supers, RVLEN=RVLEN, FC=FC, K=K,
                nch1=nch1, nch2=nch2, MIDW=MIDW)


def gpos_to_slab(g):
    """global scrambled array [NPAD] -> [128, SLAB] slab view."""
    return g.reshape(128, SLAB)


def full_numpy(inputs, lay=None, with_bf16=True):
    """End-to-end mirror: d,p,z chains + final Y/S + recursion."""
    if lay is None:
        lay = build_all(np.asarray(inputs["edges_s2t"], np.int64),
                        np.asarray(inputs["edges_t2s"], np.int64))
    supers, RVLEN, FC, MIDW = (lay[k] for k in ("supers", "RVLEN", "FC", "MIDW"))
    conv = to_bf16_f32 if with_bf16 else (lambda x: x)

    def run_chain(fins, u_global):
        """one direction pass for all cores; u_global [NPAD] f32 scrambled
        (other side's order); returns this side's outputs [NPAD] scrambled."""
        out = np.zeros(NPAD, np.float32)
        us = gpos_to_slab(conv(u_global))
        for c in range(NCORES):
            O = mirror_pass(fins[c], us, FC, MIDW, RVLEN, supers, with_bf16)
            out[c * CH:(c + 1) * CH] = O.reshape(-1)  # p*98+slot partition-major
        return out

    def d_chain(fins):
        out = np.zeros(NPAD, np.float32)
        for c in range(NCORES):
            RV = fins[c]["dmask"]
            O = np.zeros((128, NSLOT), np.float32)
            for (Lp, cnt, off, slot0) in supers:
                O[:, slot0:slot0 + cnt] = RV[:, off:off + cnt * Lp].reshape(
                    128, cnt, Lp).sum(axis=2)
            out[c * CH:(c + 1) * CH] = O.reshape(-1)
        return out

    finA, finB = lay["finA"], lay["finB"]
    d_s = d_chain(finA)
    d_t = d_chain(finB)
    p_s = run_chain(finA, d_t)
    p_t = run_chain(finB, d_s)
    z_s = run_chain(finA, p_t)
    z_t = run_chain(finB, p_s)

    # final: Y = U4 @ [X, 1] per side, in scrambled order
    xs = pack_x_scrambled(np.asarray(inputs["x_s"], np.float32), lay["posS"])
    xt = pack_x_scrambled(np.asarray(inputs["x_t"], np.float32), lay["posT"])
    mask_s = mask_scrambled(lay["posS"])
    mask_t = mask_scrambled(lay["posT"])
    Us = np.stack([mask_s, d_s * mask_s, p_s * mask_s, z_s * mask_s])
    Ut = np.stack([mask_t, d_t * mask_t, p_t * mask_t, z_t * mask_t])
    Ys = Us @ xs
    Yt = Ut @ xt
    Ss = Us.sum(1)
    St = Ut.sum(1)
    return final_recursion(Ys, Yt, Ss, St, inputs)


def pack_x_scrambled(x, pos):
    out = np.zeros((NPAD, x.shape[1]), np.float32)
    out[pos[map_ids(np.arange(NREAL))]] = x
    return out


def mask_scrambled(pos):
    m = np.zeros(NPAD, np.float32)
    m[pos[map_ids(np.arange(NREAL))]] = 1.0
    return m


def final_recursion(Ys, Yt, Ss, St, inputs):
    f64 = np.float64
    Wl_s2t = np.asarray(inputs["Wl_s2t"], f64); Wr_s2t = np.asarray(inputs["Wr_s2t"], f64)
    b_s2t = np.asarray(inputs["b_s2t"], f64)
    Wl_t2s = np.asarray(inputs["Wl_t2s"], f64); Wr_t2s = np.asarray(inputs["Wr_t2s"], f64)
    b_t2s = np.asarray(inputs["b_t2s"], f64)
    W_lin = np.asarray(inputs["W_lin"], f64); b_lin = np.asarray(inputs["b_lin"], f64)
    Ys = Ys.astype(f64); Yt = Yt.astype(f64)
    Ss = Ss.astype(f64); St = St.astype(f64)

    def term(side, u_id, r, layer):
        if layer == 0:
            Y = Ys if side == "s" else Yt
            return Y[u_id] @ r
        if side == "s":
            Wl, Wr, bb, S, other = Wl_t2s[layer-1], Wr_t2s[layer-1], b_t2s[layer-1], Ss, "t"
        else:
            Wl, Wr, bb, S, other = Wl_s2t[layer-1], Wr_s2t[layer-1], b_s2t[layer-1], St, "s"
        return (term(other, u_id + 1, Wl @ r, layer - 1)
                + S[u_id] * (bb @ r)
                + term(side, u_id, Wr @ r, layer - 1))

    r0 = W_lin[:, 0]
    tot = term("s", 0, r0, L) + term("t", 0, r0, L) + b_lin[0]
    return np.array([[tot]], dtype=np.float32)


# ---------------- device kernel ----------------
from contextlib import ExitStack
import concourse.bass as bass
import concourse.tile as tile
from concourse import bacc, mybir
from concourse.bass_utils import run_bass_kernel_spmd
from concourse.masks import make_identity
import ml_dtypes

F32 = mybir.dt.float32
BF16 = mybir.dt.bfloat16
I16 = mybir.dt.int16


def build_kernel_v2(FC, K, nch1, nch2, nchp, RVLEN, supers, reps=1,
                    scan_bf16=True, dbg=False):
    MIDW = K * 128
    nc = bacc.Bacc("TRN2", target_bir_lowering=False, debug=False,
                   num_devices=8)

    def din(name, shape, dt=F32):
        return nc.dram_tensor(name, shape, dt, kind="ExternalInput")

    ins = {}
    for D in ("A", "B"):
        ins[f"lsp{D}"] = din(f"lsp{D}", [128, nchp * SLAB], I16)
        ins[f"m{D}"] = din(f"m{D}", [128, FC], BF16 if scan_bf16 else F32)
        ins[f"ls1{D}"] = din(f"ls1{D}", [128, nch1 * FC], I16)
        ins[f"ls2{D}"] = din(f"ls2{D}", [128, nch2 * MIDW], I16)
        ins[f"dmask{D}"] = din(f"dmask{D}", [128, RVLEN], BF16)
    ins["xs"] = din("xs", [CH, 64])
    ins["xt"] = din("xt", [CH, 64])
    ins["rmask_s"] = din("rmask_s", [CH])
    ins["rmask_t"] = din("rmask_t", [CH])

    res_s = nc.dram_tensor("res_s", [4, 65], F32, kind="ExternalOutput")
    res_t = nc.dram_tensor("res_t", [4, 65], F32, kind="ExternalOutput")

    dram = {}
    dram["d_loc2"] = nc.dram_tensor("d_loc2", [2 * CH], F32)
    dram["p_loc2"] = nc.dram_tensor("p_loc2", [2 * CH], F32)
    dram["z_locA"] = nc.dram_tensor("z_locA", [CH], F32)
    dram["z_locB"] = nc.dram_tensor("z_locB", [CH], F32)
    if dbg:
        dbg_cp = {}
        for nm in ("d_loc2", "p_loc2", "z_locA", "z_locB"):
            dbg_cp[nm] = nc.dram_tensor("dbg_" + nm, list(dram[nm].shape),
                                        F32, kind="ExternalOutput")
        dbg_sc = nc.dram_tensor("dbg_sc", [128, FC], BF16,
                                kind="ExternalOutput")
        dbg_mid = nc.dram_tensor("dbg_mid", [128, MIDW], BF16,
                                 kind="ExternalOutput")
        dbg_midT = nc.dram_tensor("dbg_midT", [128, MIDW], BF16,
                                  kind="ExternalOutput")
        dbg_rv = nc.dram_tensor("dbg_rv", [128, RVLEN], BF16,
                                kind="ExternalOutput")
    dram["d_full2"] = nc.dram_tensor("d_full2", [2 * NPAD], F32,
                                     addr_space="Shared")
    dram["p_full2"] = nc.dram_tensor("p_full2", [2 * NPAD], F32,
                                     addr_space="Shared")

    with tile.TileContext(nc) as tc, ExitStack() as ctx:
        stat = ctx.enter_context(tc.tile_pool(name="stat", bufs=1))
        idxp = ctx.enter_context(tc.tile_pool(name="idxp", bufs=1))
        wb = ctx.enter_context(tc.tile_pool(name="wb", bufs=1))
        ws = ctx.enter_context(tc.tile_pool(name="ws", bufs=2))
        psum = ctx.enter_context(tc.tile_pool(name="ps", bufs=2, space="PSUM"))

        ident = stat.tile([128, 128], BF16, tag="id")
        make_identity(nc, ident[:])

        statics = {}
        for D in ("A", "B"):
            t = stat.tile([128, nchp * SLAB], I16, tag=f"lsp{D}")
            nc.sync.dma_start(t[:], ins[f"lsp{D}"].ap())
            statics[f"lsp{D}"] = t
            t = stat.tile([128, FC], BF16 if scan_bf16 else F32, tag=f"m{D}")
            nc.sync.dma_start(t[:], ins[f"m{D}"].ap())
            statics[f"m{D}"] = t
            t = stat.tile([128, RVLEN], BF16, tag=f"dm{D}")
            nc.sync.dma_start(t[:], ins[f"dmask{D}"].ap())
            statics[f"dm{D}"] = t

        def reduce_out(rv_ap_tile, out_dram, out_off, in_bf16=True):
            OUT = ws.tile([128, NSLOT], F32, tag="OUT")
            for (Lp, cnt, off_rv, slot0) in supers:
                nc.vector.tensor_reduce(
                    out=OUT[:, slot0:slot0 + cnt],
                    in_=bass.AP(rv_ap_tile.tensor, off_rv,
                                [[RVLEN, 128], [Lp, cnt], [1, Lp]]),
                    axis=mybir.AxisListType.X, op=mybir.AluOpType.add)
            nc.sync.dma_start(
                bass.AP(out_dram, out_off, [[NSLOT, 128], [1, NSLOT]]),
                OUT[:])

        def pass_dir(D, table_dram, side_off, out_dram, out_off):
            # u-slab [128, 784] f32 -> bf16
            us32 = ws.tile([128, SLAB], F32, tag="us32")
            nc.sync.dma_start(
                us32[:],
                bass.AP(table_dram, side_off,
                        [[2 * CH, 8], [SLAB, 16], [1, SLAB]]))
            us16 = ws.tile([128, SLAB], BF16, tag="us16")
            nc.vector.tensor_copy(us16[:], us32[:])
            # LS-place into EV chunks
            EV = wb.tile([128, FC], BF16, tag=f"EV{D}")
            lsp = statics[f"lsp{D}"]
            for ck in range(nchp):
                o = ck * LSMAX
                ln = min(LSMAX, FC - o)
                nc.gpsimd.local_scatter(
                    EV[:, o:o + ln], us16[:],
                    lsp[:, ck * SLAB:(ck + 1) * SLAB],
                    channels=128, num_elems=ln, num_idxs=SLAB)
            # scan-expand
            if scan_bf16:
                sc = wb.tile([128, FC], BF16, tag=f"sc{D}")
                nc.vector.tensor_tensor_scan(
                    sc[:], statics[f"m{D}"][:], EV[:], 0.0,
                    mybir.AluOpType.mult, mybir.AluOpType.add)
            else:
                EV32 = wb.tile([128, FC], F32, tag="EV32")
                nc.vector.tensor_copy(EV32[:], EV[:])
                sc32 = wb.tile([128, FC], F32, tag="sc32")
                nc.vector.tensor_tensor_scan(
                    sc32[:], statics[f"m{D}"][:], EV32[:], 0.0,
                    mybir.AluOpType.mult, mybir.AluOpType.add)
                sc = wb.tile([128, FC], BF16, tag=f"sc{D}")
                nc.vector.tensor_copy(sc[:], sc32[:])
            # LS1 -> mid
            ls1 = idxp.tile([128, nch1 * FC], I16, tag="ls1")
            nc.sync.dma_start(ls1[:], ins[f"ls1{D}"].ap())
            mid = wb.tile([128, MIDW], BF16, tag="mid")
            for ci in range(nch1):
                o = ci * BLK * 128
                ln = min(BLK * 128, MIDW - o)
                nc.gpsimd.local_scatter(
                    mid[:, o:o + ln], sc[:],
                    ls1[:, ci * FC:(ci + 1) * FC],
                    channels=128, num_elems=ln, num_idxs=FC)
            # transpose blocks (groups of 4 into one PSUM bank)
            midT = wb.tile([128, MIDW], BF16, tag="midT")
            for g in range(0, K, 4):
                nb = min(4, K - g)
                ps = psum.tile([128, 512], BF16, tag="tps")
                for bi in range(nb):
                    b = g + bi
                    nc.tensor.transpose(
                        out=ps[:, bi * 128:(bi + 1) * 128],
                        in_=mid[:, b * 128:(b + 1) * 128],
                        identity=ident[:])
                nc.vector.tensor_copy(
                    midT[:, g * 128:(g + nb) * 128], ps[:, :nb * 128])
            # LS2 -> RV
            ls2 = idxp.tile([128, nch2 * MIDW], I16, tag="ls2")
            nc.sync.dma_start(ls2[:], ins[f"ls2{D}"].ap())
            RV = wb.tile([128, RVLEN], BF16, tag="RV")
            for cj in range(nch2):
                o = cj * LSMAX
                ln = min(LSMAX, RVLEN - o)
                nc.gpsimd.local_scatter(
                    RV[:, o:o + ln], midT[:],
                    ls2[:, cj * MIDW:(cj + 1) * MIDW],
                    channels=128, num_elems=ln, num_idxs=MIDW)
            if dbg and D == "A" and out_dram is dram["p_loc2"]:
                nc.sync.dma_start(dbg_sc.ap(), sc[:])
                nc.sync.dma_start(dbg_mid.ap(), mid[:])
                nc.sync.dma_start(dbg_midT.ap(), midT[:])
                nc.sync.dma_start(dbg_rv.ap(), RV[:])
            reduce_out(RV, out_dram, out_off)

        def allgather(loc, full):
            nc.gpsimd.collective_compute(
                "AllGather", mybir.AluOpType.bypass,
                replica_groups=[list(range(8))],
                ins=[bass.AP(loc, 0, [[1, 1], [1, 2 * CH]]).opt()],
                outs=[bass.AP(full, 0, [[1, 1], [1, 2 * NPAD]]).opt()])

        for _ in range(reps):
            # d phase: reduce the static dmask
            reduce_out(statics["dmA"], dram["d_loc2"], 0)
            reduce_out(statics["dmB"], dram["d_loc2"], CH)
            allgather(dram["d_loc2"], dram["d_full2"])
            # p phase: A consumes side t (off CH), B consumes side s (off 0)
            pass_dir("A", dram["d_full2"], CH, dram["p_loc2"], 0)
            pass_dir("B", dram["d_full2"], 0, dram["p_loc2"], CH)
            allgather(dram["p_loc2"], dram["p_full2"])
            # z phase
            pass_dir("A", dram["p_full2"], CH, dram["z_locA"], 0)
            pass_dir("B", dram["p_full2"], 0, dram["z_locB"], 0)

        if dbg:
            for nm in ("d_loc2", "p_loc2", "z_locA", "z_locB"):
                n_el = dram[nm].shape[0]
                nc.sync.dma_start(
                    bass.AP(dbg_cp[nm], 0, [[1, 1], [1, n_el]]),
                    bass.AP(dram[nm], 0, [[1, 1], [1, n_el]]))

        # final: per side Y[4,65] = sum_n u4[n] * [X[n,:], 1]
        for side, xin, off, zl, rout in (
                ("s", "xs", 0, "z_locA", res_s),
                ("t", "xt", CH, "z_locB", res_t)):
            rmask_in = ins[f"rmask_{side}"]
            xr = idxp.tile([128, NSLOT, 65], F32, tag="xr")
            nc.sync.dma_start(
                bass.AP(xr.tensor, 0, [[NSLOT * 65, 128], [65, NSLOT], [1, 64]]),
                ins[xin].ap())
            nc.vector.memset(
                bass.AP(xr.tensor, 64, [[NSLOT * 65, 128], [65, NSLOT], [1, 1]]),
                1.0)
            u4 = ws.tile([128, NSLOT, 4], F32, tag="u4")
            nc.sync.dma_start(
                bass.AP(u4.tensor, 0, [[NSLOT * 4, 128], [4, NSLOT], [1, 1]]),
                rmask_in.ap())
            for i, (dr, doff) in enumerate(((dram["d_loc2"], off),
                                            (dram["p_loc2"], off),
                                            (dram[zl], 0))):
                nc.sync.dma_start(
                    bass.AP(u4.tensor, i + 1, [[NSLOT * 4, 128], [4, NSLOT], [1, 1]]),
                    bass.AP(dr, doff, [[NSLOT, 128], [1, NSLOT]]))
            ps = psum.tile([4, 65], F32, tag="fps")
            for j in range(NSLOT):
                nc.tensor.matmul(ps[:], u4[:, j, :], xr[:, j, :],
                                 start=(j == 0), stop=(j == NSLOT - 1))
            outt = ws.tile([4, 65], F32, tag="outt")
            nc.vector.tensor_copy(outt[:], ps[:])
            nc.sync.dma_start(rout.ap(), outt[:])

    nc.compile()
    return nc


def _to_bf16(x):
    return np.asarray(x, np.float32).astype(ml_dtypes.bfloat16)


def make_in_maps(lay, inputs):
    xs = pack_x_scrambled(np.asarray(inputs["x_s"], np.float32), lay["posS"])
    xt = pack_x_scrambled(np.asarray(inputs["x_t"], np.float32), lay["posT"])
    rmask_s = mask_scrambled(lay["posS"])
    rmask_t = mask_scrambled(lay["posT"])
    nchp = lay["finA"][0]["lsp"].shape[0]
    scan_bf16 = lay.get("scan_bf16", True)
    in_maps = []
    for c in range(NCORES):
        im = {}
        for D, fins in (("A", lay["finA"]), ("B", lay["finB"])):
            f = fins[c]
            im[f"lsp{D}"] = np.ascontiguousarray(
                f["lsp"].transpose(1, 0, 2).reshape(128, -1))
            im[f"m{D}"] = (_to_bf16(f["m"]) if scan_bf16
                           else np.asarray(f["m"], np.float32))
            im[f"ls1{D}"] = np.ascontiguousarray(
                f["ls1"].transpose(1, 0, 2).reshape(128, -1))
            im[f"ls2{D}"] = np.ascontiguousarray(
                f["ls2"].transpose(1, 0, 2).reshape(128, -1))
            im[f"dmask{D}"] = _to_bf16(f["dmask"])
        im["xs"] = xs[c * CH:(c + 1) * CH]
        im["xt"] = xt[c * CH:(c + 1) * CH]
        im["rmask_s"] = rmask_s[c * CH:(c + 1) * CH]
        im["rmask_t"] = rmask_t[c * CH:(c + 1) * CH]
        in_maps.append(im)
    return in_maps


_NC_CACHE = {}


def prepare_for_bench(inputs):
    lay = build_all(np.asarray(inputs["edges_s2t"], np.int64),
                    np.asarray(inputs["edges_t2s"], np.int64))
    in_maps = make_in_maps(lay, inputs)
    return dict(lay=lay, in_maps=in_maps)


def build_from_prep(prep, reps=1):
    lay = prep["lay"]
    nchp = lay["finA"][0]["lsp"].shape[0]
    return build_kernel_v2(lay["FC"], lay["K"], lay["nch1"], lay["nch2"],
                           nchp, lay["RVLEN"], lay["supers"], reps=reps)


def kernel(**inputs) -> np.ndarray:
    prep = prepare_for_bench(inputs)
    lay = prep["lay"]
    nchp = lay["finA"][0]["lsp"].shape[0]
    key = (lay["FC"], lay["K"], lay["nch1"], lay["nch2"], nchp,
           lay["RVLEN"], tuple(tuple(s) for s in lay["supers"]))
    if key not in _NC_CACHE:
        _NC_CACHE[key] = build_kernel_v2(
            lay["FC"], lay["K"], lay["nch1"], lay["nch2"], nchp,
            lay["RVLEN"], lay["supers"])
    nc = _NC_CACHE[key]
    res = run_bass_kernel_spmd(nc, prep["in_maps"], core_ids=list(range(8)),
                               trace=False)
    Ys = sum(r["res_s"] for r in res.results)
    Yt = sum(r["res_t"] for r in res.results)
    return final_recursion(Ys[:, :64], Yt[:, :64], Ys[:, 64], Yt[:, 64],
                           inputs)




# revision 7
# speedup vs baseline: 1.0843x; 1.0401x over previous
"""Self-contained Trainium2 Bass kernel for nn_BipartiteGNN (v2).

Collapsed-linear formulation: the network is fully linear, so the [1,1]
output reduces to degree-chain vectors (d = A 1, p = A d_other, z = A p_other
per side) contracted with the node features. The device computes the chains
with a local_scatter permutation network (no ap_gather):
  u-slab DMA -> LS-place -> scan-expand -> LS1 -> PE blockwise transpose ->
  LS2 -> strided tensor_reduce, all bf16 with fp32 accumulation; final
  Y = [mask,d,p,z] @ [X,1] via PE matmul; tiny weight recursion on host.
"""
import numpy as np

NCORES = 8
CH = 12544          # rows per NC (98*128)
NPAD = NCORES * CH  # 100352
NREAL = 100_000
SLAB = 784          # columns per partition (128*784 = NPAD)
NSLOT = 98          # rows per partition
LSMAX = 2046        # local_scatter dest limit (int16 elems)
BLK = 15            # mid blocks per LS1 chunk (15*128=1920 <= 2046)
L = 3


def map_ids(n):
    c = n // 12500
    return c * CH + (n - c * 12500)


def _rank_in_groups(keys, order_by):
    """rank of each element within its key group, ordered by order_by."""
    n = len(keys)
    order = np.lexsort((order_by, keys))
    ks = keys[order]
    first = np.ones(n, bool)
    first[1:] = ks[1:] != ks[:-1]
    gs = np.zeros(n, np.int64)
    idx = np.arange(n)
    gs[first] = idx[first]
    gs = np.maximum.accumulate(gs)
    rank_sorted = idx - gs
    rank = np.empty(n, np.int64)
    rank[order] = rank_sorted
    return rank


def build_scramble(rows_pad):
    """deg[NPAD], pos[NPAD] (padded id -> scrambled global pos),
    degsorted[NCORES, CH] (per-NC degrees in rank order)."""
    deg = np.bincount(rows_pad, minlength=NPAD)
    pos = np.zeros(NPAD, np.int64)
    degsorted = np.zeros((NCORES, CH), np.int64)
    for c in range(NCORES):
        d = deg[c * CH:(c + 1) * CH]
        order = np.lexsort((np.arange(CH), -d))
        rank = np.empty(CH, np.int64)
        rank[order] = np.arange(CH)
        part = rank % 128
        slot = rank // 128
        pos[c * CH:(c + 1) * CH] = c * CH + part * NSLOT + slot
        degsorted[c] = d[order]
    return deg, pos, degsorted


def rebalance_side(pos, rows_pad, cols_pad, col_pos):
    """Permute rows among the 8 partitions of each slab-group (same slot)
    to flatten the (col-partition q, row-partition p') flow matrix.
    Leaves this side's slab map (= what the other direction reads) intact:
    slab((8g+i)*98+slot) == g for any i in [0,8)."""
    q_edge = (col_pos[cols_pad] // SLAB).astype(np.int32)
    newpos = pos.copy()
    for c in range(NCORES):
        base_id = c * CH
        loc = pos[base_id:base_id + CH] - c * CH
        part = loc // NSLOT
        slot = loc % NSLOT
        grp = part // 8
        sel = np.where((rows_pad >= base_id) & (rows_pad < base_id + CH))[0]
        er = (rows_pad[sel] - base_id).astype(np.int64)
        eq = q_edge[sel]
        order = np.argsort(er, kind="stable")
        er_s = er[order]
        eq_s = eq[order]
        starts = np.searchsorted(er_s, np.arange(CH + 1))
        deg = starts[1:] - starts[:-1]
        cnt = np.zeros((16, 128, 8), np.int32)
        ordr = np.lexsort((-deg, slot, grp))
        qrows = [None] * CH
        pick_of = np.zeros(CH, np.int8)
        for rid in range(CH):
            qs = eq_s[starts[rid]:starts[rid + 1]]
            if len(qs):
                uc = np.bincount(qs, minlength=128)
                qrows[rid] = np.nonzero(uc)[0], uc
        for pass_i in range(2):
            for blk_i in range(0, CH, 8):
                ids = ordr[blk_i:blk_i + 8]
                g = grp[ids[0]]
                s_ = slot[ids[0]]
                cg = cnt[g]
                if pass_i == 1:
                    # remove this block's contributions, then re-pick all 8
                    for rid in ids:
                        if qrows[rid] is not None:
                            uq, uc = qrows[rid]
                            cg[uq, pick_of[rid]] -= uc[uq]
                used = np.zeros(8, bool)
                for rid in ids:
                    if qrows[rid] is not None:
                        uq, uc = qrows[rid]
                        sc = (cg[uq, :] + uc[uq, None]).max(axis=0)
                    else:
                        uq = None
                        sc = np.zeros(8, np.int64)
                    sc = np.where(used, 1 << 30, sc)
                    pick = int(np.argmin(sc))
                    used[pick] = True
                    pick_of[rid] = pick
                    if uq is not None:
                        cg[uq, pick] += uc[uq]
        for rid in range(CH):
            g = grp[rid]
            s_ = slot[rid]
            newpos[base_id + rid] = base_id + (8 * g + int(pick_of[rid])) * NSLOT + s_
    return newpos


def build_regions(degsorted_list):
    """Shared super-region table from per-NC/side degree-rank profiles.
    Micro-region b (slot b): L_micro[b] = max over profiles of
    max(deg[rank 128b .. 128b+127]) = deg at rank 128b (sorted desc).
    Merge consecutive micros into supers (cap padding).
    Returns list of (Lpad, cnt, rv_off, slot0) and RVLEN."""
    lm = np.zeros(NSLOT, np.int64)
    for ds in degsorted_list:
        lm = np.maximum(lm, ds[:, ::1].reshape(NCORES, NSLOT, 128).max(axis=2).max(axis=0))
    # ensure even L (AP niceness) and >=2
    supers = []
    b = 0
    while b < NSLOT:
        Lmax = lm[b]
        e = b + 1
        while e < NSLOT and e - b < 24 and lm[e] >= max(2, Lmax * 0.93):
            e += 1
        Lpad = int(max(2, Lmax + (Lmax & 1)))
        supers.append([Lpad, e - b, 0, b])
        b = e
    off = 0
    for s in supers:
        s[2] = off
        off += s[0] * s[1]
    RVLEN = off + (off & 1)
    return supers, RVLEN


def build_direction(rows_pad, cols_pad, row_pos, col_pos, supers):
    """Per-NC edge bookkeeping for one direction."""
    rpos = row_pos[rows_pad]
    nc_of = rpos // CH
    rloc = rpos - nc_of * CH
    rpart = rloc // NSLOT
    rslot = rloc - rpart * NSLOT

    cpos = col_pos[cols_pad]
    cslab = cpos // SLAB
    cwithin = cpos - cslab * SLAB

    slot_off = np.zeros(NSLOT, np.int64)
    for (Lp, cnt, off, slot0) in supers:
        for k in range(cnt):
            slot_off[slot0 + k] = off + k * Lp

    cores = []
    for c in range(NCORES):
        sel = np.where(nc_of == c)[0]
        rp = rpart[sel]
        rs = rslot[sel]
        q = cslab[sel]
        cw = cwithin[sel]
        j = _rank_in_groups(rp * NSLOT + rs, np.arange(len(sel)))
        t = slot_off[rs] + j                       # RV position within p'

        # col side: EV layout per partition q: per distinct col (ascending):
        # [start slot][edge slots]; compute ev index per edge + run starts.
        key_c = q * SLAB + cw
        order_c = np.lexsort((np.arange(len(sel)), key_c))
        kc_s = key_c[order_c]
        firstc = np.ones(len(sel), bool)
        firstc[1:] = kc_s[1:] != kc_s[:-1]
        qq = kc_s // SLAB
        nslots_sorted = firstc.astype(np.int64) + 1
        cs = np.cumsum(nslots_sorted)
        qfirst = np.ones(len(sel), bool)
        qfirst[1:] = qq[1:] != qq[:-1]
        base = np.zeros(len(sel), np.int64)
        base[qfirst] = cs[qfirst] - nslots_sorted[qfirst]
        base = np.maximum.accumulate(base)
        evpos_sorted = cs - base - 1
        ev_i = np.empty(len(sel), np.int64)
        ev_i[order_c] = evpos_sorted
        ev_len = np.zeros(128, np.int64)
        if len(sel):
            lastq = np.ones(len(sel), bool)
            lastq[:-1] = qq[1:] != qq[:-1]
            ev_len[qq[lastq]] = evpos_sorted[lastq] + 1
        rs_q = qq[firstc]
        rs_cw = kc_s[firstc] - rs_q * SLAB
        rs_pos = evpos_sorted[firstc] - 1
        cores.append(dict(rp=rp, t=t, q=q, ev_i=ev_i, ev_len=ev_len,
                          rs_q=rs_q, rs_cw=rs_cw, rs_pos=rs_pos))
    return cores


def finalize_direction(cores, FC, K, RVLEN):
    """Device arrays per core given shared sizes."""
    nch1 = (K + BLK - 1) // BLK
    MIDW = K * 128
    nch2 = int(np.ceil(RVLEN / LSMAX))
    out = []
    for co in cores:
        rp, t, q, ev_i = co["rp"], co["t"], co["q"], co["ev_i"]
        b = _rank_in_groups(q * 128 + rp, t)
        assert b.max(initial=0) < K

        # LS-place idx, chunked over EV dest (chunks of LSMAX)
        nchp = int(np.ceil(FC / LSMAX))
        lsp = -np.ones((nchp, 128, SLAB), np.int16)
        ck = co["rs_pos"] // LSMAX
        lsp[ck, co["rs_q"], co["rs_cw"]] = (co["rs_pos"] - ck * LSMAX).astype(np.int16)

        m = np.ones((128, FC), np.float32)
        m[co["rs_q"], co["rs_pos"]] = 0.0

        ls1 = -np.ones((nch1, 128, FC), np.int16)
        ci = b // BLK
        dest1 = (b - ci * BLK) * 128 + rp
        ls1[ci, q, ev_i] = dest1.astype(np.int16)

        # midT pos of edge: (rp, b*128 + q)
        ls2 = -np.ones((nch2, 128, MIDW), np.int16)
        cj = t // LSMAX
        dest2 = t - cj * LSMAX
        ls2[cj, rp, b * 128 + q] = dest2.astype(np.int16)

        dmask = np.zeros((128, RVLEN), np.float32)
        dmask[rp, t] = 1.0

        out.append(dict(lsp=lsp, m=m, ls1=ls1, ls2=ls2, dmask=dmask,
                        rp=rp, t=t, q=q, b=b, ev_i=ev_i))
    return out, nch1, nch2, MIDW


def _bf16(x):
    return x.astype(np.float32).view(np.uint32) >> 16


def to_bf16_f32(x):
    """round-to-nearest-even bf16, kept as float32."""
    x = np.asarray(x, np.float32)
    u = x.view(np.uint32)
    rounded = ((u + 0x7FFF + ((u >> 16) & 1)) & 0xFFFF0000).astype(np.uint32)
    return rounded.view(np.float32)


def mirror_pass(fin, uslab, FC, MIDW, RVLEN, supers, with_bf16=True):
    """Numpy mirror of one core's pass. uslab [128, SLAB] f32 (already the
    slab contents). Returns OUT [128, NSLOT] f32."""
    conv = to_bf16_f32 if with_bf16 else (lambda x: x)
    uslab = conv(uslab)
    EV = np.zeros((128, FC), np.float32)
    lsp = fin["lsp"].astype(np.int64)
    for ck in range(lsp.shape[0]):
        pok, sok = np.where(lsp[ck] >= 0)
        EV[pok, ck * LSMAX + lsp[ck, pok, sok]] = uslab[pok, sok]
    # segmented scan (expand): m=0 starts a new segment with value EV
    m = fin["m"]
    bidx = np.where(m == 0.0, np.arange(FC)[None, :], -1)
    bidx = np.maximum.accumulate(bidx, axis=1)
    sc = np.take_along_axis(EV, np.maximum(bidx, 0), axis=1)
    sc[bidx < 0] = 0.0
    # LS1 -> mid -> transpose -> midT  (pure permutation; emulate directly)
    midT = np.zeros((128, MIDW), np.float32)
    rp, t, q, b, ev_i = (fin[k] for k in ("rp", "t", "q", "b", "ev_i"))
    midT[rp, b * 128 + q] = sc[q, ev_i]
    RV = np.zeros((128, RVLEN), np.float32)
    RV[rp, t] = midT[rp, b * 128 + q]
    OUT = np.zeros((128, NSLOT), np.float32)
    for (Lp, cnt, off, slot0) in supers:
        seg = RV[:, off:off + cnt * Lp].reshape(128, cnt, Lp)
        OUT[:, slot0:slot0 + cnt] = seg.sum(axis=2)
    return OUT


def build_all(edges_A, edges_B):
    """edges_A = edges_s2t (rows=s=row0, cols=t=row1); edges_B = edges_t2s."""
    rowsA = map_ids(np.asarray(edges_A[0], np.int64))
    colsA = map_ids(np.asarray(edges_A[1], np.int64))
    rowsB = map_ids(np.asarray(edges_B[0], np.int64))
    colsB = map_ids(np.asarray(edges_B[1], np.int64))

    degA, posS, dsrtA = build_scramble(rowsA)   # side s scramble from A rows
    degB, posT, dsrtB = build_scramble(rowsB)   # side t scramble from B rows
    posS = rebalance_side(posS, rowsA, colsA, posT)
    posT = rebalance_side(posT, rowsB, colsB, posS)
    supers, RVLEN = build_regions([dsrtA, dsrtB])

    coresA = build_direction(rowsA, colsA, posS, posT, supers)
    coresB = build_direction(rowsB, colsB, posT, posS, supers)

    FC = 0
    for co in coresA + coresB:
        FC = max(FC, int(co["ev_len"].max()))
    FC += FC & 1
    K = 0
    for co in coresA + coresB:
        b = _rank_in_groups(co["q"] * 128 + co["rp"], co["t"])
        K = max(K, int(b.max(initial=0)) + 1)

    finA, nch1, nch2, MIDW = finalize_direction(coresA, FC, K, RVLEN)
    finB, _, _, _ = finalize_direction(coresB, FC, K, RVLEN)

    return dict(finA=finA, finB=finB, posS=posS, posT=posT,
                supers=supers, RVLEN=RVLEN, FC=FC, K=K,
                nch1=nch1, nch2=nch2, MIDW=MIDW)


def gpos_to_slab(g):
    """global scrambled array [NPAD] -> [128, SLAB] slab view."""
    return g.reshape(128, SLAB)


def full_numpy(inputs, lay=None, with_bf16=True):
    """End-to-end mirror: d,p,z chains + final Y/S + recursion."""
    if lay is None:
        lay = build_all(np.asarray(inputs["edges_s2t"], np.int64),
                        np.asarray(inputs["edges_t2s"], np.int64))
    supers, RVLEN, FC, MIDW = (lay[k] for k in ("supers", "RVLEN", "FC", "MIDW"))
    conv = to_bf16_f32 if with_bf16 else (lambda x: x)

    def run_chain(fins, u_global):
        """one direction pass for all cores; u_global [NPAD] f32 scrambled
        (other side's order); returns this side's outputs [NPAD] scrambled."""
        out = np.zeros(NPAD, np.float32)
        us = gpos_to_slab(conv(u_global))
        for c in range(NCORES):
            O = mirror_pass(fins[c], us, FC, MIDW, RVLEN, supers, with_bf16)
            out[c * CH:(c + 1) * CH] = O.reshape(-1)  # p*98+slot partition-major
        return out

    def d_chain(fins):
        out = np.zeros(NPAD, np.float32)
        for c in range(NCORES):
            RV = fins[c]["dmask"]
            O = np.zeros((128, NSLOT), np.float32)
            for (Lp, cnt, off, slot0) in supers:
                O[:, slot0:slot0 + cnt] = RV[:, off:off + cnt * Lp].reshape(
                    128, cnt, Lp).sum(axis=2)
            out[c * CH:(c + 1) * CH] = O.reshape(-1)
        return out

    finA, finB = lay["finA"], lay["finB"]
    d_s = d_chain(finA)
    d_t = d_chain(finB)
    p_s = run_chain(finA, d_t)
    p_t = run_chain(finB, d_s)
    z_s = run_chain(finA, p_t)
    z_t = run_chain(finB, p_s)

    # final: Y = U4 @ [X, 1] per side, in scrambled order
    xs = pack_x_scrambled(np.asarray(inputs["x_s"], np.float32), lay["posS"])
    xt = pack_x_scrambled(np.asarray(inputs["x_t"], np.float32), lay["posT"])
    mask_s = mask_scrambled(lay["posS"])
    mask_t = mask_scrambled(lay["posT"])
    Us = np.stack([mask_s, d_s * mask_s, p_s * mask_s, z_s * mask_s])
    Ut = np.stack([mask_t, d_t * mask_t, p_t * mask_t, z_t * mask_t])
    Ys = Us @ xs
    Yt = Ut @ xt
    Ss = Us.sum(1)
    St = Ut.sum(1)
    return final_recursion(Ys, Yt, Ss, St, inputs)


def pack_x_scrambled(x, pos):
    out = np.zeros((NPAD, x.shape[1]), np.float32)
    out[pos[map_ids(np.arange(NREAL))]] = x
    return out


def mask_scrambled(pos):
    m = np.zeros(NPAD, np.float32)
    m[pos[map_ids(np.arange(NREAL))]] = 1.0
    return m


def final_recursion(Ys, Yt, Ss, St, inputs):
    f64 = np.float64
    Wl_s2t = np.asarray(inputs["Wl_s2t"], f64); Wr_s2t = np.asarray(inputs["Wr_s2t"], f64)
    b_s2t = np.asarray(inputs["b_s2t"], f64)
    Wl_t2s = np.asarray(inputs["Wl_t2s"], f64); Wr_t2s = np.asarray(inputs["Wr_t2s"], f64)
    b_t2s = np.asarray(inputs["b_t2s"], f64)
    W_lin = np.asarray(inputs["W_lin"], f64); b_lin = np.asarray(inputs["b_lin"], f64)
    Ys = Ys.astype(f64); Yt = Yt.astype(f64)
    Ss = Ss.astype(f64); St = St.astype(f64)

    def term(side, u_id, r, layer):
        if layer == 0:
            Y = Ys if side == "s" else Yt
            return Y[u_id] @ r
        if side == "s":
            Wl, Wr, bb, S, other = Wl_t2s[layer-1], Wr_t2s[layer-1], b_t2s[layer-1], Ss, "t"
        else:
            Wl, Wr, bb, S, other = Wl_s2t[layer-1], Wr_s2t[layer-1], b_s2t[layer-1], St, "s"
        return (term(other, u_id + 1, Wl @ r, layer - 1)
                + S[u_id] * (bb @ r)
                + term(side, u_id, Wr @ r, layer - 1))

    r0 = W_lin[:, 0]
    tot = term("s", 0, r0, L) + term("t", 0, r0, L) + b_lin[0]
    return np.array([[tot]], dtype=np.float32)


# ---------------- device kernel ----------------
from contextlib import ExitStack
import concourse.bass as bass
import concourse.tile as tile
from concourse import bacc, mybir
from concourse.bass_utils import run_bass_kernel_spmd
from concourse.masks import make_identity
import ml_dtypes

F32 = mybir.dt.float32
BF16 = mybir.dt.bfloat16
I16 = mybir.dt.int16


def build_kernel_v2(FC, K, nch1, nch2, nchp, RVLEN, supers, reps=1,
                    scan_bf16=True, dbg=False):
    MIDW = K * 128
    nc = bacc.Bacc("TRN2", target_bir_lowering=False, debug=False,
                   num_devices=8)

    def din(name, shape, dt=F32):
        return nc.dram_tensor(name, shape, dt, kind="ExternalInput")

    ins = {}
    for D in ("A", "B"):
        ins[f"lsp{D}"] = din(f"lsp{D}", [128, nchp * SLAB], I16)
        ins[f"m{D}"] = din(f"m{D}", [128, FC], BF16 if scan_bf16 else F32)
        ins[f"ls1{D}"] = din(f"ls1{D}", [128, nch1 * FC], I16)
        ins[f"ls2{D}"] = din(f"ls2{D}", [128, nch2 * MIDW], I16)
        ins[f"dmask{D}"] = din(f"dmask{D}", [128, RVLEN], BF16)
    ins["xs"] = din("xs", [CH, 64])
    ins["xt"] = din("xt", [CH, 64])
    ins["rmask_s"] = din("rmask_s", [CH])
    ins["rmask_t"] = din("rmask_t", [CH])

    res_s = nc.dram_tensor("res_s", [4, 65], F32, kind="ExternalOutput")
    res_t = nc.dram_tensor("res_t", [4, 65], F32, kind="ExternalOutput")

    dram = {}
    dram["d_loc2"] = nc.dram_tensor("d_loc2", [2 * CH], F32)
    dram["p_loc2"] = nc.dram_tensor("p_loc2", [2 * CH], F32)
    dram["z_locA"] = nc.dram_tensor("z_locA", [CH], F32)
    dram["z_locB"] = nc.dram_tensor("z_locB", [CH], F32)
    if dbg:
        dbg_cp = {}
        for nm in ("d_loc2", "p_loc2", "z_locA", "z_locB"):
            dbg_cp[nm] = nc.dram_tensor("dbg_" + nm, list(dram[nm].shape),
                                        F32, kind="ExternalOutput")
        dbg_sc = nc.dram_tensor("dbg_sc", [128, FC], BF16,
                                kind="ExternalOutput")
        dbg_mid = nc.dram_tensor("dbg_mid", [128, MIDW], BF16,
                                 kind="ExternalOutput")
        dbg_midT = nc.dram_tensor("dbg_midT", [128, MIDW], BF16,
                                  kind="ExternalOutput")
        dbg_rv = nc.dram_tensor("dbg_rv", [128, RVLEN], BF16,
                                kind="ExternalOutput")
    dram["d_full2"] = nc.dram_tensor("d_full2", [2 * NPAD], F32,
                                     addr_space="Shared")
    dram["p_full2"] = nc.dram_tensor("p_full2", [2 * NPAD], F32,
                                     addr_space="Shared")

    with tile.TileContext(nc) as tc, ExitStack() as ctx:
        stat = ctx.enter_context(tc.tile_pool(name="stat", bufs=1))
        idxp = ctx.enter_context(tc.tile_pool(name="idxp", bufs=1))
        wb = ctx.enter_context(tc.tile_pool(name="wb", bufs=1))
        ws = ctx.enter_context(tc.tile_pool(name="ws", bufs=2))
        psum = ctx.enter_context(tc.tile_pool(name="ps", bufs=4, space="PSUM"))

        ident = stat.tile([128, 128], BF16, tag="id")
        make_identity(nc, ident[:])

        statics = {}
        for D in ("A", "B"):
            t = stat.tile([128, nchp * SLAB], I16, tag=f"lsp{D}")
            nc.sync.dma_start(t[:], ins[f"lsp{D}"].ap())
            statics[f"lsp{D}"] = t
            t = stat.tile([128, FC], BF16 if scan_bf16 else F32, tag=f"m{D}")
            nc.sync.dma_start(t[:], ins[f"m{D}"].ap())
            statics[f"m{D}"] = t
            t = stat.tile([128, RVLEN], BF16, tag=f"dm{D}")
            nc.sync.dma_start(t[:], ins[f"dmask{D}"].ap())
            statics[f"dm{D}"] = t

        def reduce_out(rv_ap_tile, out_dram, out_off, in_bf16=True):
            OUT = ws.tile([128, NSLOT], F32, tag="OUT")
            for (Lp, cnt, off_rv, slot0) in supers:
                nc.vector.tensor_reduce(
                    out=OUT[:, slot0:slot0 + cnt],
                    in_=bass.AP(rv_ap_tile.tensor, off_rv,
                                [[RVLEN, 128], [Lp, cnt], [1, Lp]]),
                    axis=mybir.AxisListType.X, op=mybir.AluOpType.add)
            nc.sync.dma_start(
                bass.AP(out_dram, out_off, [[NSLOT, 128], [1, NSLOT]]),
                OUT[:])

        def pass_dir(D, table_dram, side_off, out_dram, out_off):
            # index loads first: overlap DMA with place/scan stages
            ls1 = idxp.tile([128, nch1 * FC], I16, tag="ls1")
            nc.sync.dma_start(ls1[:], ins[f"ls1{D}"].ap())
            ls2 = idxp.tile([128, nch2 * MIDW], I16, tag="ls2")
            nc.sync.dma_start(ls2[:], ins[f"ls2{D}"].ap())
            # u-slab [128, 784] f32 -> bf16
            us32 = ws.tile([128, SLAB], F32, tag="us32")
            nc.sync.dma_start(
                us32[:],
                bass.AP(table_dram, side_off,
                        [[2 * CH, 8], [SLAB, 16], [1, SLAB]]))
            us16 = ws.tile([128, SLAB], BF16, tag="us16")
            nc.vector.tensor_copy(us16[:], us32[:])
            # LS-place into EV chunks
            EV = wb.tile([128, FC], BF16, tag=f"EV{D}")
            lsp = statics[f"lsp{D}"]
            for ck in range(nchp):
                o = ck * LSMAX
                ln = min(LSMAX, FC - o)
                nc.gpsimd.local_scatter(
                    EV[:, o:o + ln], us16[:],
                    lsp[:, ck * SLAB:(ck + 1) * SLAB],
                    channels=128, num_elems=ln, num_idxs=SLAB)
            # scan-expand
            if scan_bf16:
                sc = wb.tile([128, FC], BF16, tag=f"sc{D}")
                nc.vector.tensor_tensor_scan(
                    sc[:], statics[f"m{D}"][:], EV[:], 0.0,
                    mybir.AluOpType.mult, mybir.AluOpType.add)
            else:
                EV32 = wb.tile([128, FC], F32, tag="EV32")
                nc.vector.tensor_copy(EV32[:], EV[:])
                sc32 = wb.tile([128, FC], F32, tag="sc32")
                nc.vector.tensor_tensor_scan(
                    sc32[:], statics[f"m{D}"][:], EV32[:], 0.0,
                    mybir.AluOpType.mult, mybir.AluOpType.add)
                sc = wb.tile([128, FC], BF16, tag=f"sc{D}")
                nc.vector.tensor_copy(sc[:], sc32[:])
            # LS1 -> mid
            mid = wb.tile([128, MIDW], BF16, tag="mid")
            for ci in range(nch1):
                o = ci * BLK * 128
                ln = min(BLK * 128, MIDW - o)
                nc.gpsimd.local_scatter(
                    mid[:, o:o + ln], sc[:],
                    ls1[:, ci * FC:(ci + 1) * FC],
                    channels=128, num_elems=ln, num_idxs=FC)
            # transpose blocks (groups of 4 into one PSUM bank)
            midT = wb.tile([128, MIDW], BF16, tag="midT")
            for g in range(0, K, 4):
                nb = min(4, K - g)
                ps = psum.tile([128, 512], BF16, tag="tps")
                for bi in range(nb):
                    b = g + bi
                    nc.tensor.transpose(
                        out=ps[:, bi * 128:(bi + 1) * 128],
                        in_=mid[:, b * 128:(b + 1) * 128],
                        identity=ident[:])
                nc.vector.tensor_copy(
                    midT[:, g * 128:(g + nb) * 128], ps[:, :nb * 128])
            # LS2 -> RV
            RV = wb.tile([128, RVLEN], BF16, tag="RV")
            for cj in range(nch2):
                o = cj * LSMAX
                ln = min(LSMAX, RVLEN - o)
                nc.gpsimd.local_scatter(
                    RV[:, o:o + ln], midT[:],
                    ls2[:, cj * MIDW:(cj + 1) * MIDW],
                    channels=128, num_elems=ln, num_idxs=MIDW)
            if dbg and D == "A" and out_dram is dram["p_loc2"]:
                nc.sync.dma_start(dbg_sc.ap(), sc[:])
                nc.sync.dma_start(dbg_mid.ap(), mid[:])
                nc.sync.dma_start(dbg_midT.ap(), midT[:])
                nc.sync.dma_start(dbg_rv.ap(), RV[:])
            reduce_out(RV, out_dram, out_off)

        def allgather(loc, full):
            nc.gpsimd.collective_compute(
                "AllGather", mybir.AluOpType.bypass,
                replica_groups=[list(range(8))],
                ins=[bass.AP(loc, 0, [[1, 1], [1, 2 * CH]]).opt()],
                outs=[bass.AP(full, 0, [[1, 1], [1, 2 * NPAD]]).opt()])

        for _ in range(reps):
            # d phase: reduce the static dmask
            reduce_out(statics["dmA"], dram["d_loc2"], 0)
            reduce_out(statics["dmB"], dram["d_loc2"], CH)
            allgather(dram["d_loc2"], dram["d_full2"])
            # p phase: A consumes side t (off CH), B consumes side s (off 0)
            pass_dir("A", dram["d_full2"], CH, dram["p_loc2"], 0)
            pass_dir("B", dram["d_full2"], 0, dram["p_loc2"], CH)
            allgather(dram["p_loc2"], dram["p_full2"])
            # z phase
            pass_dir("A", dram["p_full2"], CH, dram["z_locA"], 0)
            pass_dir("B", dram["p_full2"], 0, dram["z_locB"], 0)

        if dbg:
            for nm in ("d_loc2", "p_loc2", "z_locA", "z_locB"):
                n_el = dram[nm].shape[0]
                nc.sync.dma_start(
                    bass.AP(dbg_cp[nm], 0, [[1, 1], [1, n_el]]),
                    bass.AP(dram[nm], 0, [[1, 1], [1, n_el]]))

        # final: per side Y[4,65] = sum_n u4[n] * [X[n,:], 1]
        for side, xin, off, zl, rout in (
                ("s", "xs", 0, "z_locA", res_s),
                ("t", "xt", CH, "z_locB", res_t)):
            rmask_in = ins[f"rmask_{side}"]
            xr = idxp.tile([128, NSLOT, 65], F32, tag="xr")
            nc.sync.dma_start(
                bass.AP(xr.tensor, 0, [[NSLOT * 65, 128], [65, NSLOT], [1, 64]]),
                ins[xin].ap())
            nc.vector.memset(
                bass.AP(xr.tensor, 64, [[NSLOT * 65, 128], [65, NSLOT], [1, 1]]),
                1.0)
            u4 = ws.tile([128, NSLOT, 4], F32, tag="u4")
            nc.sync.dma_start(
                bass.AP(u4.tensor, 0, [[NSLOT * 4, 128], [4, NSLOT], [1, 1]]),
                rmask_in.ap())
            for i, (dr, doff) in enumerate(((dram["d_loc2"], off),
                                            (dram["p_loc2"], off),
                                            (dram[zl], 0))):
                nc.sync.dma_start(
                    bass.AP(u4.tensor, i + 1, [[NSLOT * 4, 128], [4, NSLOT], [1, 1]]),
                    bass.AP(dr, doff, [[NSLOT, 128], [1, NSLOT]]))
            ps = psum.tile([4, 65], F32, tag="fps")
            for j in range(NSLOT):
                nc.tensor.matmul(ps[:], u4[:, j, :], xr[:, j, :],
                                 start=(j == 0), stop=(j == NSLOT - 1))
            outt = ws.tile([4, 65], F32, tag="outt")
            nc.vector.tensor_copy(outt[:], ps[:])
            nc.sync.dma_start(rout.ap(), outt[:])

    nc.compile()
    return nc


def _to_bf16(x):
    return np.asarray(x, np.float32).astype(ml_dtypes.bfloat16)


def make_in_maps(lay, inputs):
    xs = pack_x_scrambled(np.asarray(inputs["x_s"], np.float32), lay["posS"])
    xt = pack_x_scrambled(np.asarray(inputs["x_t"], np.float32), lay["posT"])
    rmask_s = mask_scrambled(lay["posS"])
    rmask_t = mask_scrambled(lay["posT"])
    nchp = lay["finA"][0]["lsp"].shape[0]
    scan_bf16 = lay.get("scan_bf16", True)
    in_maps = []
    for c in range(NCORES):
        im = {}
        for D, fins in (("A", lay["finA"]), ("B", lay["finB"])):
            f = fins[c]
            im[f"lsp{D}"] = np.ascontiguousarray(
                f["lsp"].transpose(1, 0, 2).reshape(128, -1))
            im[f"m{D}"] = (_to_bf16(f["m"]) if scan_bf16
                           else np.asarray(f["m"], np.float32))
            im[f"ls1{D}"] = np.ascontiguousarray(
                f["ls1"].transpose(1, 0, 2).reshape(128, -1))
            im[f"ls2{D}"] = np.ascontiguousarray(
                f["ls2"].transpose(1, 0, 2).reshape(128, -1))
            im[f"dmask{D}"] = _to_bf16(f["dmask"])
        im["xs"] = xs[c * CH:(c + 1) * CH]
        im["xt"] = xt[c * CH:(c + 1) * CH]
        im["rmask_s"] = rmask_s[c * CH:(c + 1) * CH]
        im["rmask_t"] = rmask_t[c * CH:(c + 1) * CH]
        in_maps.append(im)
    return in_maps


_NC_CACHE = {}


def prepare_for_bench(inputs):
    lay = build_all(np.asarray(inputs["edges_s2t"], np.int64),
                    np.asarray(inputs["edges_t2s"], np.int64))
    in_maps = make_in_maps(lay, inputs)
    return dict(lay=lay, in_maps=in_maps)


def build_from_prep(prep, reps=1):
    lay = prep["lay"]
    nchp = lay["finA"][0]["lsp"].shape[0]
    return build_kernel_v2(lay["FC"], lay["K"], lay["nch1"], lay["nch2"],
                           nchp, lay["RVLEN"], lay["supers"], reps=reps)


def kernel(**inputs) -> np.ndarray:
    prep = prepare_for_bench(inputs)
    lay = prep["lay"]
    nchp = lay["finA"][0]["lsp"].shape[0]
    key = (lay["FC"], lay["K"], lay["nch1"], lay["nch2"], nchp,
           lay["RVLEN"], tuple(tuple(s) for s in lay["supers"]))
    if key not in _NC_CACHE:
        _NC_CACHE[key] = build_kernel_v2(
            lay["FC"], lay["K"], lay["nch1"], lay["nch2"], nchp,
            lay["RVLEN"], lay["supers"])
    nc = _NC_CACHE[key]
    res = run_bass_kernel_spmd(nc, prep["in_maps"], core_ids=list(range(8)),
                               trace=False)
    Ys = sum(r["res_s"] for r in res.results)
    Yt = sum(r["res_t"] for r in res.results)
    return final_recursion(Ys[:, :64], Yt[:, :64], Ys[:, 64], Yt[:, 64],
                           inputs)




# revision 8
# speedup vs baseline: 2.3454x; 2.1630x over previous
"""Self-contained Trainium2 Bass kernel for nn_BipartiteGNN (v2).

Collapsed-linear formulation: the network is fully linear, so the [1,1]
output reduces to degree-chain vectors (d = A 1, p = A d_other, z = A p_other
per side) contracted with the node features. The device computes the chains
with a local_scatter permutation network (no ap_gather):
  u-slab DMA -> LS-place -> scan-expand -> LS1 -> PE blockwise transpose ->
  LS2 -> strided tensor_reduce, all bf16 with fp32 accumulation; final
  Y = [mask,d,p,z] @ [X,1] via PE matmul; tiny weight recursion on host.
"""
import numpy as np

NCORES = 8
CH = 12544          # rows per NC (98*128)
NPAD = NCORES * CH  # 100352
NREAL = 100_000
SLAB = 784          # columns per partition (128*784 = NPAD)
NSLOT = 98          # rows per partition
LSMAX = 2046        # local_scatter dest limit (int16 elems)
BLK = 15            # mid blocks per LS1 chunk (15*128=1920 <= 2046)
L = 3


def map_ids(n):
    c = n // 12500
    return c * CH + (n - c * 12500)


def _rank_in_groups(keys, order_by):
    """rank of each element within its key group, ordered by order_by."""
    n = len(keys)
    order = np.lexsort((order_by, keys))
    ks = keys[order]
    first = np.ones(n, bool)
    first[1:] = ks[1:] != ks[:-1]
    gs = np.zeros(n, np.int64)
    idx = np.arange(n)
    gs[first] = idx[first]
    gs = np.maximum.accumulate(gs)
    rank_sorted = idx - gs
    rank = np.empty(n, np.int64)
    rank[order] = rank_sorted
    return rank


def build_scramble(rows_pad):
    """deg[NPAD], pos[NPAD] (padded id -> scrambled global pos),
    degsorted[NCORES, CH] (per-NC degrees in rank order)."""
    deg = np.bincount(rows_pad, minlength=NPAD)
    pos = np.zeros(NPAD, np.int64)
    degsorted = np.zeros((NCORES, CH), np.int64)
    for c in range(NCORES):
        d = deg[c * CH:(c + 1) * CH]
        order = np.lexsort((np.arange(CH), -d))
        rank = np.empty(CH, np.int64)
        rank[order] = np.arange(CH)
        part = rank % 128
        slot = rank // 128
        pos[c * CH:(c + 1) * CH] = c * CH + part * NSLOT + slot
        degsorted[c] = d[order]
    return deg, pos, degsorted


def rebalance_side(pos, rows_pad, cols_pad, col_pos):
    """Permute rows among the 8 partitions of each slab-group (same slot)
    to flatten the (col-partition q, row-partition p') flow matrix.
    Leaves this side's slab map (= what the other direction reads) intact:
    slab((8g+i)*98+slot) == g for any i in [0,8)."""
    q_edge = (col_pos[cols_pad] // SLAB).astype(np.int32)
    newpos = pos.copy()
    for c in range(NCORES):
        base_id = c * CH
        loc = pos[base_id:base_id + CH] - c * CH
        part = loc // NSLOT
        slot = loc % NSLOT
        grp = part // 8
        sel = np.where((rows_pad >= base_id) & (rows_pad < base_id + CH))[0]
        er = (rows_pad[sel] - base_id).astype(np.int64)
        eq = q_edge[sel]
        order = np.argsort(er, kind="stable")
        er_s = er[order]
        eq_s = eq[order]
        starts = np.searchsorted(er_s, np.arange(CH + 1))
        deg = starts[1:] - starts[:-1]
        cnt = np.zeros((16, 128, 8), np.int32)
        ordr = np.lexsort((-deg, slot, grp))
        qrows = [None] * CH
        pick_of = np.zeros(CH, np.int8)
        for rid in range(CH):
            qs = eq_s[starts[rid]:starts[rid + 1]]
            if len(qs):
                uc = np.bincount(qs, minlength=128)
                qrows[rid] = np.nonzero(uc)[0], uc
        for pass_i in range(2):
            for blk_i in range(0, CH, 8):
                ids = ordr[blk_i:blk_i + 8]
                g = grp[ids[0]]
                s_ = slot[ids[0]]
                cg = cnt[g]
                if pass_i == 1:
                    # remove this block's contributions, then re-pick all 8
                    for rid in ids:
                        if qrows[rid] is not None:
                            uq, uc = qrows[rid]
                            cg[uq, pick_of[rid]] -= uc[uq]
                used = np.zeros(8, bool)
                for rid in ids:
                    if qrows[rid] is not None:
                        uq, uc = qrows[rid]
                        sc = (cg[uq, :] + uc[uq, None]).max(axis=0)
                    else:
                        uq = None
                        sc = np.zeros(8, np.int64)
                    sc = np.where(used, 1 << 30, sc)
                    pick = int(np.argmin(sc))
                    used[pick] = True
                    pick_of[rid] = pick
                    if uq is not None:
                        cg[uq, pick] += uc[uq]
        for rid in range(CH):
            g = grp[rid]
            s_ = slot[rid]
            newpos[base_id + rid] = base_id + (8 * g + int(pick_of[rid])) * NSLOT + s_
    return newpos


def build_regions(degsorted_list):
    """Shared super-region table from per-NC/side degree-rank profiles.
    Micro-region b (slot b): L_micro[b] = max over profiles of
    max(deg[rank 128b .. 128b+127]) = deg at rank 128b (sorted desc).
    Merge consecutive micros into supers (cap padding).
    Returns list of (Lpad, cnt, rv_off, slot0) and RVLEN."""
    lm = np.zeros(NSLOT, np.int64)
    for ds in degsorted_list:
        lm = np.maximum(lm, ds[:, ::1].reshape(NCORES, NSLOT, 128).max(axis=2).max(axis=0))
    # ensure even L (AP niceness) and >=2
    supers = []
    b = 0
    while b < NSLOT:
        Lmax = lm[b]
        e = b + 1
        while e < NSLOT and e - b < 24 and lm[e] >= max(2, Lmax * 0.93):
            e += 1
        Lpad = int(max(2, Lmax + (Lmax & 1)))
        supers.append([Lpad, e - b, 0, b])
        b = e
    off = 0
    for s in supers:
        s[2] = off
        off += s[0] * s[1]
    RVLEN = off + (off & 1)
    return supers, RVLEN


def build_direction(rows_pad, cols_pad, row_pos, col_pos, supers):
    """Per-NC edge bookkeeping for one direction."""
    rpos = row_pos[rows_pad]
    nc_of = rpos // CH
    rloc = rpos - nc_of * CH
    rpart = rloc // NSLOT
    rslot = rloc - rpart * NSLOT

    cpos = col_pos[cols_pad]
    cslab = cpos // SLAB
    cwithin = cpos - cslab * SLAB

    slot_off = np.zeros(NSLOT, np.int64)
    for (Lp, cnt, off, slot0) in supers:
        for k in range(cnt):
            slot_off[slot0 + k] = off + k * Lp

    cores = []
    for c in range(NCORES):
        sel = np.where(nc_of == c)[0]
        rp = rpart[sel]
        rs = rslot[sel]
        q = cslab[sel]
        cw = cwithin[sel]
        j = _rank_in_groups(rp * NSLOT + rs, np.arange(len(sel)))
        t = slot_off[rs] + j                       # RV position within p'

        # col side: EV layout per partition q: per distinct col (ascending):
        # [start slot][edge slots]; compute ev index per edge + run starts.
        key_c = q * SLAB + cw
        order_c = np.lexsort((np.arange(len(sel)), key_c))
        kc_s = key_c[order_c]
        firstc = np.ones(len(sel), bool)
        firstc[1:] = kc_s[1:] != kc_s[:-1]
        qq = kc_s // SLAB
        nslots_sorted = firstc.astype(np.int64) + 1
        cs = np.cumsum(nslots_sorted)
        qfirst = np.ones(len(sel), bool)
        qfirst[1:] = qq[1:] != qq[:-1]
        base = np.zeros(len(sel), np.int64)
        base[qfirst] = cs[qfirst] - nslots_sorted[qfirst]
        base = np.maximum.accumulate(base)
        evpos_sorted = cs - base - 1
        ev_i = np.empty(len(sel), np.int64)
        ev_i[order_c] = evpos_sorted
        ev_len = np.zeros(128, np.int64)
        if len(sel):
            lastq = np.ones(len(sel), bool)
            lastq[:-1] = qq[1:] != qq[:-1]
            ev_len[qq[lastq]] = evpos_sorted[lastq] + 1
        rs_q = qq[firstc]
        rs_cw = kc_s[firstc] - rs_q * SLAB
        rs_pos = evpos_sorted[firstc] - 1
        cores.append(dict(rp=rp, t=t, q=q, ev_i=ev_i, ev_len=ev_len,
                          rs_q=rs_q, rs_cw=rs_cw, rs_pos=rs_pos))
    return cores


def finalize_direction(cores, FC, K, RVLEN):
    """Device arrays per core given shared sizes."""
    nch1 = (K + BLK - 1) // BLK
    MIDW = K * 128
    nch2 = int(np.ceil(RVLEN / LSMAX))
    out = []
    for co in cores:
        rp, t, q, ev_i = co["rp"], co["t"], co["q"], co["ev_i"]
        b = _rank_in_groups(q * 128 + rp, t)
        assert b.max(initial=0) < K

        # LS-place idx, chunked over EV dest (chunks of LSMAX)
        nchp = int(np.ceil(FC / LSMAX))
        lsp = -np.ones((nchp, 128, SLAB), np.int16)
        ck = co["rs_pos"] // LSMAX
        lsp[ck, co["rs_q"], co["rs_cw"]] = (co["rs_pos"] - ck * LSMAX).astype(np.int16)

        m = np.ones((128, FC), np.float32)
        m[co["rs_q"], co["rs_pos"]] = 0.0

        ls1 = -np.ones((nch1, 128, FC), np.int16)
        ci = b // BLK
        dest1 = (b - ci * BLK) * 128 + rp
        ls1[ci, q, ev_i] = dest1.astype(np.int16)

        # midT pos of edge: (rp, b*128 + q)
        ls2 = -np.ones((nch2, 128, MIDW), np.int16)
        cj = t // LSMAX
        dest2 = t - cj * LSMAX
        ls2[cj, rp, b * 128 + q] = dest2.astype(np.int16)

        dmask = np.zeros((128, RVLEN), np.float32)
        dmask[rp, t] = 1.0

        out.append(dict(lsp=lsp, m=m, ls1=ls1, ls2=ls2, dmask=dmask,
                        rp=rp, t=t, q=q, b=b, ev_i=ev_i))
    return out, nch1, nch2, MIDW


def _bf16(x):
    return x.astype(np.float32).view(np.uint32) >> 16


def to_bf16_f32(x):
    """round-to-nearest-even bf16, kept as float32."""
    x = np.asarray(x, np.float32)
    u = x.view(np.uint32)
    rounded = ((u + 0x7FFF + ((u >> 16) & 1)) & 0xFFFF0000).astype(np.uint32)
    return rounded.view(np.float32)


def mirror_pass(fin, uslab, FC, MIDW, RVLEN, supers, with_bf16=True):
    """Numpy mirror of one core's pass. uslab [128, SLAB] f32 (already the
    slab contents). Returns OUT [128, NSLOT] f32."""
    conv = to_bf16_f32 if with_bf16 else (lambda x: x)
    uslab = conv(uslab)
    EV = np.zeros((128, FC), np.float32)
    lsp = fin["lsp"].astype(np.int64)
    for ck in range(lsp.shape[0]):
        pok, sok = np.where(lsp[ck] >= 0)
        EV[pok, ck * LSMAX + lsp[ck, pok, sok]] = uslab[pok, sok]
    # segmented scan (expand): m=0 starts a new segment with value EV
    m = fin["m"]
    bidx = np.where(m == 0.0, np.arange(FC)[None, :], -1)
    bidx = np.maximum.accumulate(bidx, axis=1)
    sc = np.take_along_axis(EV, np.maximum(bidx, 0), axis=1)
    sc[bidx < 0] = 0.0
    # LS1 -> mid -> transpose -> midT  (pure permutation; emulate directly)
    midT = np.zeros((128, MIDW), np.float32)
    rp, t, q, b, ev_i = (fin[k] for k in ("rp", "t", "q", "b", "ev_i"))
    midT[rp, b * 128 + q] = sc[q, ev_i]
    RV = np.zeros((128, RVLEN), np.float32)
    RV[rp, t] = midT[rp, b * 128 + q]
    OUT = np.zeros((128, NSLOT), np.float32)
    for (Lp, cnt, off, slot0) in supers:
        seg = RV[:, off:off + cnt * Lp].reshape(128, cnt, Lp)
        OUT[:, slot0:slot0 + cnt] = seg.sum(axis=2)
    return OUT


def build_all(edges_A, edges_B):
    """edges_A = edges_s2t (rows=s=row0, cols=t=row1); edges_B = edges_t2s."""
    rowsA = map_ids(np.asarray(edges_A[0], np.int64))
    colsA = map_ids(np.asarray(edges_A[1], np.int64))
    rowsB = map_ids(np.asarray(edges_B[0], np.int64))
    colsB = map_ids(np.asarray(edges_B[1], np.int64))

    degA, posS, dsrtA = build_scramble(rowsA)   # side s scramble from A rows
    degB, posT, dsrtB = build_scramble(rowsB)   # side t scramble from B rows
    posS = rebalance_side(posS, rowsA, colsA, posT)
    posT = rebalance_side(posT, rowsB, colsB, posS)
    supers, RVLEN = build_regions([dsrtA, dsrtB])

    coresA = build_direction(rowsA, colsA, posS, posT, supers)
    coresB = build_direction(rowsB, colsB, posT, posS, supers)

    FC = 0
    for co in coresA + coresB:
        FC = max(FC, int(co["ev_len"].max()))
    FC += FC & 1
    K = 0
    for co in coresA + coresB:
        b = _rank_in_groups(co["q"] * 128 + co["rp"], co["t"])
        K = max(K, int(b.max(initial=0)) + 1)

    finA, nch1, nch2, MIDW = finalize_direction(coresA, FC, K, RVLEN)
    finB, _, _, _ = finalize_direction(coresB, FC, K, RVLEN)

    return dict(finA=finA, finB=finB, posS=posS, posT=posT,
                supers=supers, RVLEN=RVLEN, FC=FC, K=K,
                nch1=nch1, nch2=nch2, MIDW=MIDW)


def gpos_to_slab(g):
    """global scrambled array [NPAD] -> [128, SLAB] slab view."""
    return g.reshape(128, SLAB)


def full_numpy(inputs, lay=None, with_bf16=True):
    """End-to-end mirror: d,p,z chains + final Y/S + recursion."""
    if lay is None:
        lay = build_all(np.asarray(inputs["edges_s2t"], np.int64),
                        np.asarray(inputs["edges_t2s"], np.int64))
    supers, RVLEN, FC, MIDW = (lay[k] for k in ("supers", "RVLEN", "FC", "MIDW"))
    conv = to_bf16_f32 if with_bf16 else (lambda x: x)

    def run_chain(fins, u_global):
        """one direction pass for all cores; u_global [NPAD] f32 scrambled
        (other side's order); returns this side's outputs [NPAD] scrambled."""
        out = np.zeros(NPAD, np.float32)
        us = gpos_to_slab(conv(u_global))
        for c in range(NCORES):
            O = mirror_pass(fins[c], us, FC, MIDW, RVLEN, supers, with_bf16)
            out[c * CH:(c + 1) * CH] = O.reshape(-1)  # p*98+slot partition-major
        return out

    def d_chain(fins):
        out = np.zeros(NPAD, np.float32)
        for c in range(NCORES):
            RV = fins[c]["dmask"]
            O = np.zeros((128, NSLOT), np.float32)
            for (Lp, cnt, off, slot0) in supers:
                O[:, slot0:slot0 + cnt] = RV[:, off:off + cnt * Lp].reshape(
                    128, cnt, Lp).sum(axis=2)
            out[c * CH:(c + 1) * CH] = O.reshape(-1)
        return out

    finA, finB = lay["finA"], lay["finB"]
    d_s = d_chain(finA)
    d_t = d_chain(finB)
    p_s = run_chain(finA, d_t)
    p_t = run_chain(finB, d_s)
    z_s = run_chain(finA, p_t)
    z_t = run_chain(finB, p_s)

    # final: Y = U4 @ [X, 1] per side, in scrambled order
    xs = pack_x_scrambled(np.asarray(inputs["x_s"], np.float32), lay["posS"])
    xt = pack_x_scrambled(np.asarray(inputs["x_t"], np.float32), lay["posT"])
    mask_s = mask_scrambled(lay["posS"])
    mask_t = mask_scrambled(lay["posT"])
    Us = np.stack([mask_s, d_s * mask_s, p_s * mask_s, z_s * mask_s])
    Ut = np.stack([mask_t, d_t * mask_t, p_t * mask_t, z_t * mask_t])
    Ys = Us @ xs
    Yt = Ut @ xt
    Ss = Us.sum(1)
    St = Ut.sum(1)
    return final_recursion(Ys, Yt, Ss, St, inputs)


def pack_x_scrambled(x, pos):
    out = np.zeros((NPAD, x.shape[1]), np.float32)
    out[pos[map_ids(np.arange(NREAL))]] = x
    return out


def mask_scrambled(pos):
    m = np.zeros(NPAD, np.float32)
    m[pos[map_ids(np.arange(NREAL))]] = 1.0
    return m


def final_recursion(Ys, Yt, Ss, St, inputs):
    f64 = np.float64
    Wl_s2t = np.asarray(inputs["Wl_s2t"], f64); Wr_s2t = np.asarray(inputs["Wr_s2t"], f64)
    b_s2t = np.asarray(inputs["b_s2t"], f64)
    Wl_t2s = np.asarray(inputs["Wl_t2s"], f64); Wr_t2s = np.asarray(inputs["Wr_t2s"], f64)
    b_t2s = np.asarray(inputs["b_t2s"], f64)
    W_lin = np.asarray(inputs["W_lin"], f64); b_lin = np.asarray(inputs["b_lin"], f64)
    Ys = Ys.astype(f64); Yt = Yt.astype(f64)
    Ss = Ss.astype(f64); St = St.astype(f64)

    def term(side, u_id, r, layer):
        if layer == 0:
            Y = Ys if side == "s" else Yt
            return Y[u_id] @ r
        if side == "s":
            Wl, Wr, bb, S, other = Wl_t2s[layer-1], Wr_t2s[layer-1], b_t2s[layer-1], Ss, "t"
        else:
            Wl, Wr, bb, S, other = Wl_s2t[layer-1], Wr_s2t[layer-1], b_s2t[layer-1], St, "s"
        return (term(other, u_id + 1, Wl @ r, layer - 1)
                + S[u_id] * (bb @ r)
                + term(side, u_id, Wr @ r, layer - 1))

    r0 = W_lin[:, 0]
    tot = term("s", 0, r0, L) + term("t", 0, r0, L) + b_lin[0]
    return np.array([[tot]], dtype=np.float32)


# ---------------- device kernel ----------------
from contextlib import ExitStack
import concourse.bass as bass
import concourse.tile as tile
from concourse import bacc, mybir
from concourse.bass_utils import run_bass_kernel_spmd
from concourse.masks import make_identity
import ml_dtypes

F32 = mybir.dt.float32
BF16 = mybir.dt.bfloat16
I16 = mybir.dt.int16


def build_kernel_v2(FC, K, nch1, nch2, nchp, RVLEN, supers, reps=1,
                    scan_bf16=True, dbg=False):
    MIDW = K * 128
    nc = bacc.Bacc("TRN2", target_bir_lowering=False, debug=False,
                   num_devices=8)

    def din(name, shape, dt=F32):
        return nc.dram_tensor(name, shape, dt, kind="ExternalInput")

    ins = {}
    for D in ("A", "B"):
        ins[f"lsp{D}"] = din(f"lsp{D}", [128, nchp * SLAB], I16)
        ins[f"m{D}"] = din(f"m{D}", [128, FC], BF16 if scan_bf16 else F32)
        ins[f"ls1{D}"] = din(f"ls1{D}", [128, nch1 * FC], I16)
        ins[f"ls2{D}"] = din(f"ls2{D}", [128, nch2 * MIDW], I16)
        ins[f"dmask{D}"] = din(f"dmask{D}", [128, RVLEN], BF16)
    ins["xs"] = din("xs", [CH, 64])
    ins["xt"] = din("xt", [CH, 64])
    ins["rmask_s"] = din("rmask_s", [CH])
    ins["rmask_t"] = din("rmask_t", [CH])

    res_s = nc.dram_tensor("res_s", [4, 65], F32, kind="ExternalOutput")
    res_t = nc.dram_tensor("res_t", [4, 65], F32, kind="ExternalOutput")

    dram = {}
    dram["d_loc2"] = nc.dram_tensor("d_loc2", [2 * CH], F32)
    dram["p_loc2"] = nc.dram_tensor("p_loc2", [2 * CH], F32)
    dram["z_locA"] = nc.dram_tensor("z_locA", [CH], F32)
    dram["z_locB"] = nc.dram_tensor("z_locB", [CH], F32)
    if dbg:
        dbg_cp = {}
        for nm in ("d_loc2", "p_loc2", "z_locA", "z_locB"):
            dbg_cp[nm] = nc.dram_tensor("dbg_" + nm, list(dram[nm].shape),
                                        F32, kind="ExternalOutput")
        dbg_sc = nc.dram_tensor("dbg_sc", [128, FC], BF16,
                                kind="ExternalOutput")
        dbg_mid = nc.dram_tensor("dbg_mid", [128, MIDW], BF16,
                                 kind="ExternalOutput")
        dbg_midT = nc.dram_tensor("dbg_midT", [128, MIDW], BF16,
                                  kind="ExternalOutput")
        dbg_rv = nc.dram_tensor("dbg_rv", [128, RVLEN], BF16,
                                kind="ExternalOutput")
    for nm in ("d_fullS", "d_fullT", "p_fullS", "p_fullT"):
        dram[nm] = nc.dram_tensor(nm, [NPAD], F32, addr_space="Shared")

    with tile.TileContext(nc) as tc, ExitStack() as ctx:
        stat = ctx.enter_context(tc.tile_pool(name="stat", bufs=1))
        idxp = ctx.enter_context(tc.tile_pool(name="idxp", bufs=1))
        wb = ctx.enter_context(tc.tile_pool(name="wb", bufs=1))
        ws = ctx.enter_context(tc.tile_pool(name="ws", bufs=2))
        psum = ctx.enter_context(tc.tile_pool(name="ps", bufs=4, space="PSUM"))

        ident = stat.tile([128, 128], BF16, tag="id")
        make_identity(nc, ident[:])

        statics = {}
        for D in ("A", "B"):
            t = stat.tile([128, nchp * SLAB], I16, tag=f"lsp{D}")
            nc.sync.dma_start(t[:], ins[f"lsp{D}"].ap())
            statics[f"lsp{D}"] = t
            t = stat.tile([128, FC], BF16 if scan_bf16 else F32, tag=f"m{D}")
            nc.sync.dma_start(t[:], ins[f"m{D}"].ap())
            statics[f"m{D}"] = t
            t = stat.tile([128, RVLEN], BF16, tag=f"dm{D}")
            nc.sync.dma_start(t[:], ins[f"dmask{D}"].ap())
            statics[f"dm{D}"] = t

        def reduce_out(rv_ap_tile, out_dram, out_off, in_bf16=True):
            OUT = ws.tile([128, NSLOT], F32, tag="OUT")
            for (Lp, cnt, off_rv, slot0) in supers:
                nc.vector.tensor_reduce(
                    out=OUT[:, slot0:slot0 + cnt],
                    in_=bass.AP(rv_ap_tile.tensor, off_rv,
                                [[RVLEN, 128], [Lp, cnt], [1, Lp]]),
                    axis=mybir.AxisListType.X, op=mybir.AluOpType.add)
            nc.sync.dma_start(
                bass.AP(out_dram, out_off, [[NSLOT, 128], [1, NSLOT]]),
                OUT[:])

        def pass_dir(D, table_dram, side_off, out_dram, out_off):
            # index loads first: overlap DMA with place/scan stages
            ls1 = idxp.tile([128, nch1 * FC], I16, tag="ls1")
            nc.sync.dma_start(ls1[:], ins[f"ls1{D}"].ap())
            ls2 = idxp.tile([128, nch2 * MIDW], I16, tag="ls2")
            nc.sync.dma_start(ls2[:], ins[f"ls2{D}"].ap())
            # u-slab [128, 784] f32 -> bf16
            us32 = ws.tile([128, SLAB], F32, tag="us32")
            nc.sync.dma_start(
                us32[:],
                bass.AP(table_dram, 0,
                        [[CH, 8], [SLAB, 16], [1, SLAB]]))
            us16 = ws.tile([128, SLAB], BF16, tag="us16")
            nc.vector.tensor_copy(us16[:], us32[:])
            # LS-place into EV chunks
            EV = wb.tile([128, FC], BF16, tag=f"EV{D}")
            lsp = statics[f"lsp{D}"]
            for ck in range(nchp):
                o = ck * LSMAX
                ln = min(LSMAX, FC - o)
                nc.gpsimd.local_scatter(
                    EV[:, o:o + ln], us16[:],
                    lsp[:, ck * SLAB:(ck + 1) * SLAB],
                    channels=128, num_elems=ln, num_idxs=SLAB)
            # scan-expand
            if scan_bf16:
                sc = wb.tile([128, FC], BF16, tag=f"sc{D}")
                nc.vector.tensor_tensor_scan(
                    sc[:], statics[f"m{D}"][:], EV[:], 0.0,
                    mybir.AluOpType.mult, mybir.AluOpType.add)
            else:
                EV32 = wb.tile([128, FC], F32, tag="EV32")
                nc.vector.tensor_copy(EV32[:], EV[:])
                sc32 = wb.tile([128, FC], F32, tag="sc32")
                nc.vector.tensor_tensor_scan(
                    sc32[:], statics[f"m{D}"][:], EV32[:], 0.0,
                    mybir.AluOpType.mult, mybir.AluOpType.add)
                sc = wb.tile([128, FC], BF16, tag=f"sc{D}")
                nc.vector.tensor_copy(sc[:], sc32[:])
            # LS1 -> mid
            mid = wb.tile([128, MIDW], BF16, tag="mid")
            for ci in range(nch1):
                o = ci * BLK * 128
                ln = min(BLK * 128, MIDW - o)
                nc.gpsimd.local_scatter(
                    mid[:, o:o + ln], sc[:],
                    ls1[:, ci * FC:(ci + 1) * FC],
                    channels=128, num_elems=ln, num_idxs=FC)
            # transpose blocks (groups of 4 into one PSUM bank)
            midT = wb.tile([128, MIDW], BF16, tag="midT")
            for g in range(0, K, 4):
                nb = min(4, K - g)
                ps = psum.tile([128, 512], BF16, tag="tps")
                for bi in range(nb):
                    b = g + bi
                    nc.tensor.transpose(
                        out=ps[:, bi * 128:(bi + 1) * 128],
                        in_=mid[:, b * 128:(b + 1) * 128],
                        identity=ident[:])
                nc.vector.tensor_copy(
                    midT[:, g * 128:(g + nb) * 128], ps[:, :nb * 128])
            # LS2 -> RV
            RV = wb.tile([128, RVLEN], BF16, tag="RV")
            for cj in range(nch2):
                o = cj * LSMAX
                ln = min(LSMAX, RVLEN - o)
                nc.gpsimd.local_scatter(
                    RV[:, o:o + ln], midT[:],
                    ls2[:, cj * MIDW:(cj + 1) * MIDW],
                    channels=128, num_elems=ln, num_idxs=MIDW)
            if dbg and D == "A" and out_dram is dram["p_loc2"]:
                nc.sync.dma_start(dbg_sc.ap(), sc[:])
                nc.sync.dma_start(dbg_mid.ap(), mid[:])
                nc.sync.dma_start(dbg_midT.ap(), midT[:])
                nc.sync.dma_start(dbg_rv.ap(), RV[:])
            reduce_out(RV, out_dram, out_off)

        def allgather(loc, off, full):
            nc.gpsimd.collective_compute(
                "AllGather", mybir.AluOpType.bypass,
                replica_groups=[list(range(8))],
                ins=[bass.AP(loc, off, [[1, 1], [1, CH]]).opt()],
                outs=[bass.AP(full, 0, [[1, 1], [1, NPAD]]).opt()])

        for _ in range(reps):
            # d phase: each side's AllGather overlaps the other's reduce
            reduce_out(statics["dmA"], dram["d_loc2"], 0)
            allgather(dram["d_loc2"], 0, dram["d_fullS"])
            reduce_out(statics["dmB"], dram["d_loc2"], CH)
            allgather(dram["d_loc2"], CH, dram["d_fullT"])
            # p phase: B first (needs d_s, gathered earliest); its output
            # p_t gathers while A's p-pass runs
            pass_dir("B", dram["d_fullS"], 0, dram["p_loc2"], CH)
            allgather(dram["p_loc2"], CH, dram["p_fullT"])
            pass_dir("A", dram["d_fullT"], 0, dram["p_loc2"], 0)
            allgather(dram["p_loc2"], 0, dram["p_fullS"])
            # z phase: A first (needs p_t), then B (needs p_s, whose
            # AllGather overlapped A's z-pass)
            pass_dir("A", dram["p_fullT"], 0, dram["z_locA"], 0)
            pass_dir("B", dram["p_fullS"], 0, dram["z_locB"], 0)

        if dbg:
            for nm in ("d_loc2", "p_loc2", "z_locA", "z_locB"):
                n_el = dram[nm].shape[0]
                nc.sync.dma_start(
                    bass.AP(dbg_cp[nm], 0, [[1, 1], [1, n_el]]),
                    bass.AP(dram[nm], 0, [[1, 1], [1, n_el]]))

        # final: per side Y[4,65] = sum_n u4[n] * [X[n,:], 1]
        for side, xin, off, zl, rout in (
                ("s", "xs", 0, "z_locA", res_s),
                ("t", "xt", CH, "z_locB", res_t)):
            rmask_in = ins[f"rmask_{side}"]
            xr = idxp.tile([128, NSLOT, 65], F32, tag="xr")
            nc.sync.dma_start(
                bass.AP(xr.tensor, 0, [[NSLOT * 65, 128], [65, NSLOT], [1, 64]]),
                ins[xin].ap())
            nc.vector.memset(
                bass.AP(xr.tensor, 64, [[NSLOT * 65, 128], [65, NSLOT], [1, 1]]),
                1.0)
            u4 = ws.tile([128, NSLOT, 4], F32, tag="u4")
            nc.sync.dma_start(
                bass.AP(u4.tensor, 0, [[NSLOT * 4, 128], [4, NSLOT], [1, 1]]),
                rmask_in.ap())
            for i, (dr, doff) in enumerate(((dram["d_loc2"], off),
                                            (dram["p_loc2"], off),
                                            (dram[zl], 0))):
                nc.sync.dma_start(
                    bass.AP(u4.tensor, i + 1, [[NSLOT * 4, 128], [4, NSLOT], [1, 1]]),
                    bass.AP(dr, doff, [[NSLOT, 128], [1, NSLOT]]))
            ps = psum.tile([4, 65], F32, tag="fps")
            for j in range(NSLOT):
                nc.tensor.matmul(ps[:], u4[:, j, :], xr[:, j, :],
                                 start=(j == 0), stop=(j == NSLOT - 1))
            outt = ws.tile([4, 65], F32, tag="outt")
            nc.vector.tensor_copy(outt[:], ps[:])
            nc.sync.dma_start(rout.ap(), outt[:])

    nc.compile()
    return nc


def _to_bf16(x):
    return np.asarray(x, np.float32).astype(ml_dtypes.bfloat16)


def make_in_maps(lay, inputs):
    xs = pack_x_scrambled(np.asarray(inputs["x_s"], np.float32), lay["posS"])
    xt = pack_x_scrambled(np.asarray(inputs["x_t"], np.float32), lay["posT"])
    rmask_s = mask_scrambled(lay["posS"])
    rmask_t = mask_scrambled(lay["posT"])
    nchp = lay["finA"][0]["lsp"].shape[0]
    scan_bf16 = lay.get("scan_bf16", True)
    in_maps = []
    for c in range(NCORES):
        im = {}
        for D, fins in (("A", lay["finA"]), ("B", lay["finB"])):
            f = fins[c]
            im[f"lsp{D}"] = np.ascontiguousarray(
                f["lsp"].transpose(1, 0, 2).reshape(128, -1))
            im[f"m{D}"] = (_to_bf16(f["m"]) if scan_bf16
                           else np.asarray(f["m"], np.float32))
            im[f"ls1{D}"] = np.ascontiguousarray(
                f["ls1"].transpose(1, 0, 2).reshape(128, -1))
            im[f"ls2{D}"] = np.ascontiguousarray(
                f["ls2"].transpose(1, 0, 2).reshape(128, -1))
            im[f"dmask{D}"] = _to_bf16(f["dmask"])
        im["xs"] = xs[c * CH:(c + 1) * CH]
        im["xt"] = xt[c * CH:(c + 1) * CH]
        im["rmask_s"] = rmask_s[c * CH:(c + 1) * CH]
        im["rmask_t"] = rmask_t[c * CH:(c + 1) * CH]
        in_maps.append(im)
    return in_maps


_NC_CACHE = {}


def prepare_for_bench(inputs):
    lay = build_all(np.asarray(inputs["edges_s2t"], np.int64),
                    np.asarray(inputs["edges_t2s"], np.int64))
    in_maps = make_in_maps(lay, inputs)
    return dict(lay=lay, in_maps=in_maps)


def build_from_prep(prep, reps=1):
    lay = prep["lay"]
    nchp = lay["finA"][0]["lsp"].shape[0]
    return build_kernel_v2(lay["FC"], lay["K"], lay["nch1"], lay["nch2"],
                           nchp, lay["RVLEN"], lay["supers"], reps=reps)


def kernel(**inputs) -> np.ndarray:
    prep = prepare_for_bench(inputs)
    lay = prep["lay"]
    nchp = lay["finA"][0]["lsp"].shape[0]
    key = (lay["FC"], lay["K"], lay["nch1"], lay["nch2"], nchp,
           lay["RVLEN"], tuple(tuple(s) for s in lay["supers"]))
    if key not in _NC_CACHE:
        _NC_CACHE[key] = build_kernel_v2(
            lay["FC"], lay["K"], lay["nch1"], lay["nch2"], nchp,
            lay["RVLEN"], lay["supers"])
    nc = _NC_CACHE[key]
    res = run_bass_kernel_spmd(nc, prep["in_maps"], core_ids=list(range(8)),
                               trace=False)
    Ys = sum(r["res_s"] for r in res.results)
    Yt = sum(r["res_t"] for r in res.results)
    return final_recursion(Ys[:, :64], Yt[:, :64], Ys[:, 64], Yt[:, 64],
                           inputs)


